# revision 1
# baseline (speedup 1.0000x reference)
"""GatedSlotAttention2 Trainium2 Bass kernel.

Sharding: 2 heads per core x 8 cores (H=16). Each core computes its two
heads' full pipeline (projections -> short conv -> chunked gated-slot scan
-> RMSNorm-gate -> partial Wo matmul); host sums the 8 partial outputs.

Scan algorithm: chunk-parallel reformulation of the per-step recurrence
with chunk size C=64 (validated vs the sequential reference to ~6e-7 in
f32; bf16 projections give ~4e-3).
"""
import numpy as np
import ml_dtypes

import concourse.bass as bass
import concourse.bacc as bacc_mod
import concourse.mybir as mybir
import concourse.tile as tile
from concourse.bass_utils import run_bass_kernel_spmd

F32 = mybir.dt.float32
BF16 = mybir.dt.bfloat16
AF = mybir.ActivationFunctionType
ALU = mybir.AluOpType
MS = bass.MemorySpace

B, T, HID = 1, 1024, 2048
H, DK, DV, M, KW = 16, 128, 128, 128, 4
SCALE = DK ** -0.5
EPS = 1e-5
C = 64            # chunk length
NCH = T // C      # 16 chunks
NKT = HID // 128  # 16 contraction tiles
HL = 2            # heads per core

_CACHE = {}


def _build_nc():
    nc = bacc_mod.Bacc("TRN2")

    # ---------------- DRAM I/O ----------------
    d_xt = nc.dram_tensor("xt", [HID, T], BF16, kind="ExternalInput")        # X^T
    d_wq = nc.dram_tensor("wq", [HID, HL * DK], BF16, kind="ExternalInput")
    d_wk = nc.dram_tensor("wk", [HID, HL * DK], BF16, kind="ExternalInput")
    d_wv = nc.dram_tensor("wv", [HID, HL * DV], BF16, kind="ExternalInput")
    d_ww = nc.dram_tensor("ww", [HID, HL * M], BF16, kind="ExternalInput")
    d_wf1 = nc.dram_tensor("wf1", [HID, DV], BF16, kind="ExternalInput")
    d_wg1 = nc.dram_tensor("wg1", [HID, DV], BF16, kind="ExternalInput")
    d_wb = nc.dram_tensor("wb", [HID, HL], BF16, kind="ExternalInput")
    d_wf2 = nc.dram_tensor("wf2", [DV, HL * M], F32, kind="ExternalInput")
    d_wg2 = nc.dram_tensor("wg2", [DV, HL * DV], F32, kind="ExternalInput")
    d_bg2 = nc.dram_tensor("bg2", [1, HL * DV], F32, kind="ExternalInput")
    d_wo = nc.dram_tensor("wo", [HL * DV, HID], BF16, kind="ExternalInput")  # norm_w folded
    d_cq = nc.dram_tensor("cq", [128, HL, KW], F32, kind="ExternalInput")
    d_ck = nc.dram_tensor("ck", [128, HL, KW], F32, kind="ExternalInput")
    d_cv = nc.dram_tensor("cv", [128, HL, KW], F32, kind="ExternalInput")
    # constants
    d_trineg = nc.dram_tensor("trineg", [C, C], F32, kind="ExternalInput")       # -1 if j<=i
    d_trirev = nc.dram_tensor("trirev", [C, C], F32, kind="ExternalInput")       # -1 if j>i
    d_negc31 = nc.dram_tensor("negc31", [C, C], F32, kind="ExternalInput")       # -1 if j<=31
    d_maskS = nc.dram_tensor("masks", [C, C], F32, kind="ExternalInput")         # SCALE if j<=i
    d_maskJ = nc.dram_tensor("maskj", [C, C], mybir.dt.uint8, kind="ExternalInput")         # 1 if j<=i
    d_negones = nc.dram_tensor("negones", [C, 128], F32, kind="ExternalInput")   # all -1
    d_ident = nc.dram_tensor("ident", [128, 128], F32, kind="ExternalInput")
    d_ones1 = nc.dram_tensor("ones1", [1, C], F32, kind="ExternalInput")         # ones row

    d_out = nc.dram_tensor("out", [T, HID], F32, kind="ExternalOutput")

    with tile.TileContext(nc) as tc:
        with (
            tc.tile_pool(name="persist", bufs=1) as pp,
            tc.tile_pool(name="wpool", bufs=2) as wp,
            tc.tile_pool(name="convT", bufs=2) as cvp,
            tc.tile_pool(name="xpad", bufs=2) as xpp,
            tc.tile_pool(name="scr", bufs=2) as scr,
            tc.tile_pool(name="ps_proj", bufs=2, space=MS.PSUM) as ps_proj,
            tc.tile_pool(name="ps_scan", bufs=4, space=MS.PSUM) as ps_scan,
            tc.tile_pool(name="ps_out", bufs=2, space=MS.PSUM) as ps_out,
        ):
            # ---------- constants to SBUF ----------
            def load_const(dram, shape, dtype=F32):
                t = pp.tile(shape, dtype, tag=dram.name + "_sb")
                nc.sync.dma_start(t[:], dram[:])
                return t

            c_trineg = load_const(d_trineg, [C, C])
            c_trirev = load_const(d_trirev, [C, C])
            c_negc31 = load_const(d_negc31, [C, C])
            c_maskS = load_const(d_maskS, [C, C])
            c_maskJ = load_const(d_maskJ, [C, C], mybir.dt.uint8)
            c_negones = load_const(d_negones, [C, 128])
            c_ident = load_const(d_ident, [128, 128])
            c_ones1 = load_const(d_ones1, [1, C])
            c_wf2 = load_const(d_wf2, [DV, HL * M])
            c_wg2 = load_const(d_wg2, [DV, HL * DV])
            c_bg2 = load_const(d_bg2, [1, HL * DV])
            c_cq = load_const(d_cq, [128, HL, KW])
            c_ck = load_const(d_ck, [128, HL, KW])
            c_cv = load_const(d_cv, [128, HL, KW])
            c_eps6 = pp.tile([C, 1], F32, tag="c_eps6")
            nc.vector.memset(c_eps6[:], 1e-6)
            c_eps5 = pp.tile([C, 1], F32, tag="c_eps5")
            nc.vector.memset(c_eps5[:], EPS)

            # ---------- X^T stream tiles + big weights ----------
            xt_sb = pp.tile([128, NKT, T], BF16, tag="xt_sb")
            xtr = d_xt.rearrange("(k p) t -> k p t", p=128)
            for kt in range(NKT):
                nc.sync.dma_start(xt_sb[:, kt, :], xtr[kt])

            wo_sb = pp.tile([128, HL, HID], BF16, tag="wo_sb")
            wor = d_wo.rearrange("(h p) o -> h p o", p=128)
            for h in range(HL):
                nc.sync.dma_start(wo_sb[:, h, :], wor[h])

            # ---------- projections + conv + silu ----------
            # conv outputs, [channel, t] layout; q/k persist, v/w rotate
            qT = pp.tile([128, HL, T], F32, tag="qT")
            kT = pp.tile([128, HL, T], F32, tag="kT")

            def project_convT(d_w, c_cw, out_tile, name):
                """out[ct][c,t] = silu(conv1d(W[:,c].T @ X^T, cw)) per c-tile."""
                w_sb = wp.tile([128, NKT, HL * 128], BF16, tag="w_load")
                wr = d_w.rearrange("(k p) c -> k p c", p=128)
                for kt in range(NKT):
                    nc.sync.dma_start(w_sb[:, kt, :], wr[kt])
                for ct in range(HL):
                    acc = [None, None]
                    for tt in range(2):
                        ps = ps_proj.tile([128, 512], F32, tag="pp")
                        for kt in range(NKT):
                            nc.tensor.matmul(
                                ps[:],
                                w_sb[:, kt, ct * 128:(ct + 1) * 128],
                                xt_sb[:, kt, tt * 512:(tt + 1) * 512],
                                start=(kt == 0), stop=(kt == NKT - 1),
                            )
                        acc[tt] = ps
                    xpad = xpp.tile([128, T + KW - 1], F32, tag="xpad")
                    nc.vector.memset(xpad[:, 0:KW - 1], 0.0)
                    for tt in range(2):
                        nc.vector.tensor_copy(
                            xpad[:, KW - 1 + tt * 512: KW - 1 + (tt + 1) * 512],
                            acc[tt][:])
                    cacc = xpp.tile([128, T], F32, tag="convacc")
                    nc.vector.tensor_scalar_mul(
                        cacc[:], xpad[:, 0:T], c_cw[:, ct, 0:1])
                    for i in range(1, KW):
                        nc.vector.scalar_tensor_tensor(
                            cacc[:], xpad[:, i:i + T], c_cw[:, ct, i:i + 1],
                            cacc[:], op0=ALU.mult, op1=ALU.add)
                    se = xpp.tile([128, T], F32, tag="se")
                    nc.scalar.activation(se[:], cacc[:], AF.Exp, scale=-1.0)
                    nc.vector.tensor_scalar_add(se[:], se[:], 1.0)
                    nc.vector.reciprocal(se[:], se[:])
                    nc.vector.tensor_mul(out_tile[:, ct, :], cacc[:], se[:])

            project_convT(d_wq, c_cq, qT, "q")
            project_convT(d_wk, c_ck, kT, "k")
            vT = cvp.tile([128, HL, T], F32, tag="convT")
            project_convT(d_wv, c_cv, vT, "v")
            wT = cvp.tile([128, HL, T], F32, tag="convT")
            project_convT(d_ww, c_cv, wT, "w")

            # ---------- gate-path projections: F1T, G1T, betaT ----------
            def proj128T(d_w, tag):
                out = pp.tile([128, T], F32, tag=tag)
                w_sb = wp.tile([128, NKT, 128], BF16, tag="w_load")
                wr = d_w.rearrange("(k p) c -> k p c", p=128)
                for kt in range(NKT):
                    nc.sync.dma_start(w_sb[:, kt, :], wr[kt])
                for tt in range(2):
                    ps = ps_proj.tile([128, 512], F32, tag="pp")
                    for kt in range(NKT):
                        nc.tensor.matmul(
                            ps[:], w_sb[:, kt, :],
                            xt_sb[:, kt, tt * 512:(tt + 1) * 512],
                            start=(kt == 0), stop=(kt == NKT - 1))
                    nc.scalar.copy(out[:, tt * 512:(tt + 1) * 512], ps[:])
                return out

            f1T = proj128T(d_wf1, "f1T")
            g1T = proj128T(d_wg1, "g1T")

            betaT = pp.tile([HL, T], F32, tag="betaT")
            wb_sb = wp.tile([128, NKT, HL], BF16, tag="wb_load")
            wbr = d_wb.rearrange("(k p) c -> k p c", p=128)
            for kt in range(NKT):
                nc.sync.dma_start(wb_sb[:, kt, :], wbr[kt])
            for tt in range(2):
                ps = ps_proj.tile([HL, 512], F32, tag="pp")
                for kt in range(NKT):
                    nc.tensor.matmul(
                        ps[:], wb_sb[:, kt, :],
                        xt_sb[:, kt, tt * 512:(tt + 1) * 512],
                        start=(kt == 0), stop=(kt == NKT - 1))
                bsl = betaT[:, tt * 512:(tt + 1) * 512]
                nc.scalar.activation(bsl, ps[:], AF.Exp, scale=-1.0)
                nc.vector.tensor_scalar_add(bsl, bsl, 1.0)
                nc.vector.reciprocal(bsl, bsl)

            # ---------- states ----------
            Sk = [pp.tile([DK, M], F32, name=f"Sk{h}", tag=f"Sk{h}") for h in range(HL)]
            Sv = [pp.tile([M, DV], F32, name=f"Sv{h}", tag=f"Sv{h}") for h in range(HL)]
            for h in range(HL):
                nc.vector.memset(Sk[h][:], 0.0)
                nc.vector.memset(Sv[h][:], 0.0)

            oT = [pp.tile([DV, NCH, C], BF16, name=f"oT{h}", tag=f"oT{h}") for h in range(HL)]

            # ---------- chunked scan ----------
            for n in range(NCH):
                t0 = n * C
                # shared across the two heads: gpos/gate/beta for this chunk
                gps = ps_scan.tile([C, HL * M], F32, tag="ps")
                nc.tensor.matmul(gps[:], f1T[:, t0:t0 + C], c_wf2[:],
                                 start=True, stop=True)
                gpos = scr.tile([C, HL * M], F32, tag="gpos")
                nc.scalar.activation(gpos[:], gps[:], AF.Exp, scale=-1.0)
                nc.scalar.activation(gpos[:], gpos[:], AF.Ln, bias=1.0)

                gt_ps = ps_scan.tile([C, HL * DV], F32, tag="ps")
                nc.tensor.matmul(gt_ps[:], g1T[:, t0:t0 + C], c_wg2[:],
                                 start=True, stop=False)
                nc.tensor.matmul(gt_ps[:], c_ones1[:], c_bg2[:],
                                 start=False, stop=True)
                gate = scr.tile([C, HL * DV], F32, tag="gate")
                nc.scalar.activation(gate[:], gt_ps[:], AF.Exp, scale=-1.0)
                nc.vector.tensor_scalar_add(gate[:], gate[:], 1.0)
                nc.vector.reciprocal(gate[:], gate[:])

                bt_ps = ps_scan.tile([C, HL], F32, tag="ps")
                nc.tensor.transpose(bt_ps[:], betaT[:, t0:t0 + C],
                                    c_ident[0:HL, 0:HL])
                beta = scr.tile([C, HL], F32, tag="beta")
                nc.scalar.copy(beta[:], bt_ps[:])

                for h in range(HL):
                    hs = slice(h * 128, (h + 1) * 128)
                    # --- per-chunk transposes: K, V, W ---
                    kps = ps_scan.tile([C, 128], F32, tag="ps")
                    nc.tensor.transpose(kps[:], kT[:, h, t0:t0 + C], c_ident[:])
                    Kc = scr.tile([C, 128], F32, tag="Kc")
                    nc.scalar.copy(Kc[:], kps[:])

                    vps = ps_scan.tile([C, 128], F32, tag="ps")
                    nc.tensor.transpose(vps[:], vT[:, h, t0:t0 + C], c_ident[:])
                    Vc = scr.tile([C, 128], F32, tag="Vc")
                    nc.scalar.copy(Vc[:], vps[:])

                    wps = ps_scan.tile([C, 128], F32, tag="ps")
                    nc.tensor.transpose(wps[:], wT[:, h, t0:t0 + C], c_ident[:])
                    # l2norm + beta scaling -> bw
                    w2 = scr.tile([C, 128], F32, tag="w2")
                    ss = scr.tile([C, 1], F32, tag="ss")
                    nc.scalar.activation(w2[:], wps[:], AF.Square, accum_out=ss[:])
                    sd = scr.tile([C, 1], F32, tag="sd")
                    nc.scalar.activation(sd[:], ss[:], AF.Ln, bias=c_eps6[:])
                    rs = scr.tile([C, 1], F32, tag="rs")
                    nc.scalar.activation(rs[:], sd[:], AF.Exp, scale=-0.5)
                    rsb = scr.tile([C, 1], F32, tag="rsb")
                    nc.vector.tensor_mul(rsb[:], rs[:], beta[:, h:h + 1])
                    bw = scr.tile([C, 128], F32, tag="bw")
                    nc.vector.tensor_scalar_mul(bw[:], wps[:], rsb[:])

                    # --- gate cumsums (via triangular matmuls) ---
                    gsl = gpos[:, hs]
                    gc_ps = ps_scan.tile([C, M], F32, tag="ps")
                    nc.tensor.matmul(gc_ps[:], c_trineg[:], gsl,
                                     start=True, stop=True)
                    Gc = scr.tile([C, M], F32, tag="Gc")
                    nc.scalar.copy(Gc[:], gc_ps[:])
                    grev_ps = ps_scan.tile([C, M], F32, tag="ps")
                    nc.tensor.matmul(grev_ps[:], c_trirev[:], gsl,
                                     start=True, stop=True)
                    b1_ps = ps_scan.tile([C, M], F32, tag="ps")
                    nc.tensor.matmul(b1_ps[:], c_negc31[:], gsl,
                                     start=True, stop=True)
                    Gcp = scr.tile([C, M], F32, tag="Gcp")
                    nc.vector.tensor_sub(Gcp[:], Gc[:], b1_ps[:])
                    Lam = scr.tile([C, M], F32, tag="Lam")
                    nc.scalar.activation(Lam[:], Gc[:], AF.Exp)
                    Epos = scr.tile([C, M], F32, tag="Epos")
                    nc.scalar.activation(Epos[:], Gcp[:], AF.Exp)
                    Enege = scr.tile([C, M], F32, tag="Enege")
                    nc.scalar.activation(Enege[:], Gcp[:], AF.Exp, scale=-1.0)
                    Eneg = scr.tile([C, M], F32, tag="Eneg")
                    nc.vector.tensor_mul(Eneg[:], Enege[:], bw[:])
                    Ereve = scr.tile([C, M], F32, tag="Ereve")
                    nc.scalar.activation(Ereve[:], grev_ps[:], AF.Exp)
                    Kdec = scr.tile([C, M], F32, tag="Kdec")
                    nc.vector.tensor_mul(Kdec[:], Ereve[:], bw[:])

                    # chunk-end decay broadcasts
                    lcb_ps = ps_scan.tile([128, M], F32, tag="ps")
                    nc.tensor.matmul(lcb_ps[:], c_negones[:], gsl,
                                     start=True, stop=True)
                    LamCb = scr.tile([128, M], F32, tag="LamCb")
                    nc.scalar.activation(LamCb[:], lcb_ps[:], AF.Exp)
                    lcc_ps = ps_scan.tile([M, 1], F32, tag="ps")
                    nc.tensor.matmul(lcc_ps[:], gsl, c_negones[:, 0:1],
                                     start=True, stop=True)
                    LamCc = scr.tile([M, 1], F32, tag="LamCc")
                    nc.scalar.activation(LamCc[:], lcc_ps[:], AF.Exp)

                    # --- pass A: scores + softmax ---
                    pt_ps = ps_scan.tile([C, C], F32, tag="ps")
                    nc.tensor.matmul(pt_ps[:], kT[:, h, t0:t0 + C],
                                     qT[:, h, t0:t0 + C], start=True, stop=True)
                    Ptm = scr.tile([C, C], F32, tag="Ptm")
                    nc.vector.tensor_mul(Ptm[:], pt_ps[:], c_maskS[:])
                    intra_ps = ps_scan.tile([C, M], F32, tag="ps")
                    nc.tensor.matmul(intra_ps[:], Ptm[:], Eneg[:],
                                     start=True, stop=True)
                    qs_ps = ps_scan.tile([C, M], F32, tag="ps")
                    nc.tensor.matmul(qs_ps[:], qT[:, h, t0:t0 + C], Sk[h][:],
                                     start=True, stop=True)
                    s1 = scr.tile([C, M], F32, tag="s1")
                    nc.vector.scalar_tensor_tensor(
                        s1[:], qs_ps[:], SCALE, Lam[:],
                        op0=ALU.mult, op1=ALU.mult)
                    s2 = scr.tile([C, M], F32, tag="s2")
                    nc.vector.tensor_mul(s2[:], intra_ps[:], Epos[:])
                    sS = scr.tile([C, M], F32, tag="sS")
                    nc.vector.tensor_add(sS[:], s1[:], s2[:])
                    mx = scr.tile([C, 1], F32, tag="mx")
                    nc.vector.tensor_reduce(mx[:], sS[:], mybir.AxisListType.X,
                                            ALU.max)
                    nmx = scr.tile([C, 1], F32, tag="nmx")
                    nc.vector.tensor_scalar_mul(nmx[:], mx[:], -1.0)
                    pexp = scr.tile([C, M], F32, tag="pexp")
                    den = scr.tile([C, 1], F32, tag="den")
                    nc.scalar.activation(pexp[:], sS[:], AF.Exp, bias=nmx[:],
                                         accum_out=den[:])
                    rec = scr.tile([C, 1], F32, tag="rec")
                    nc.vector.reciprocal(rec[:], den[:])
                    aL = scr.tile([C, M], F32, tag="aL")
                    nc.vector.scalar_tensor_tensor(
                        aL[:], pexp[:], rec[:], Lam[:],
                        op0=ALU.mult, op1=ALU.mult)
                    aE = scr.tile([C, M], F32, tag="aE")
                    nc.vector.scalar_tensor_tensor(
                        aE[:], pexp[:], rec[:], Epos[:],
                        op0=ALU.mult, op1=ALU.mult)

                    # --- pass B: output ---
                    alt_ps = ps_scan.tile([M, C], F32, tag="ps")
                    nc.tensor.transpose(alt_ps[:], aL[:], c_ident[0:C, 0:C])
                    aLT = scr.tile([M, C], F32, tag="aLT")
                    nc.scalar.copy(aLT[:], alt_ps[:])
                    aet_ps = ps_scan.tile([M, C], F32, tag="ps")
                    nc.tensor.transpose(aet_ps[:], aE[:], c_ident[0:C, 0:C])
                    aET = scr.tile([M, C], F32, tag="aET")
                    nc.scalar.copy(aET[:], aet_ps[:])
                    ent_ps = ps_scan.tile([M, C], F32, tag="ps")
                    nc.tensor.transpose(ent_ps[:], Eneg[:], c_ident[0:C, 0:C])
                    EnegT = scr.tile([M, C], F32, tag="EnegT")
                    nc.scalar.copy(EnegT[:], ent_ps[:])

                    rt_ps = ps_scan.tile([C, C], F32, tag="ps")
                    nc.tensor.matmul(rt_ps[:], EnegT[:], aET[:],
                                     start=True, stop=True)
                    Rmt = scr.tile([C, C], F32, tag="Rmt")
                    nc.vector.memset(Rmt[:], 0.0)
                    nc.vector.copy_predicated(Rmt[:], c_maskJ[:], rt_ps[:])

                    o_ps = ps_scan.tile([C, DV], F32, tag="ps")
                    nc.tensor.matmul(o_ps[:], aLT[:], Sv[h][:],
                                     start=True, stop=False)
                    nc.tensor.matmul(o_ps[:], Rmt[:], Vc[:],
                                     start=False, stop=True)

                    # --- state updates ---
                    skk_ps = ps_scan.tile([DK, M], F32, tag="ps")
                    nc.tensor.matmul(skk_ps[:], Kc[:], Kdec[:],
                                     start=True, stop=True)
                    skt = scr.tile([DK, M], F32, tag="skt")
                    nc.vector.tensor_mul(skt[:], Sk[h][:], LamCb[:])
                    nc.vector.tensor_add(Sk[h][:], skt[:], skk_ps[:])
                    svk_ps = ps_scan.tile([M, DV], F32, tag="ps")
                    nc.tensor.matmul(svk_ps[:], Kdec[:], Vc[:],
                                     start=True, stop=True)
                    svt = scr.tile([M, DV], F32, tag="svt")
                    nc.vector.tensor_scalar_mul(svt[:], Sv[h][:], LamCc[:])
                    nc.vector.tensor_add(Sv[h][:], svt[:], svk_ps[:])

                    # --- epilogue: RMSNorm * sigmoid(gate), transpose ---
                    o2 = scr.tile([C, DV], F32, tag="o2")
                    oss = scr.tile([C, 1], F32, tag="oss")
                    nc.scalar.activation(o2[:], o_ps[:], AF.Square,
                                         accum_out=oss[:])
                    orm = scr.tile([C, 1], F32, tag="orm")
                    nc.scalar.activation(orm[:], oss[:], AF.Ln,
                                         scale=1.0 / DV, bias=c_eps5[:])
                    orr = scr.tile([C, 1], F32, tag="orr")
                    nc.scalar.activation(orr[:], orm[:], AF.Exp, scale=-0.5)
                    o1 = scr.tile([C, DV], F32, tag="o1")
                    nc.vector.tensor_mul(o1[:], o_ps[:], gate[:, hs])
                    of = scr.tile([C, DV], F32, tag="of")
                    nc.vector.tensor_scalar_mul(of[:], o1[:], orr[:])
                    ot_ps = ps_scan.tile([DV, C], F32, tag="ps")
                    nc.tensor.transpose(ot_ps[:], of[:], c_ident[0:C, 0:C])
                    nc.scalar.copy(oT[h][:, n, :], ot_ps[:])

            # ---------- output projection (partial; host sums cores) ----------
            for tt in range(8):
                for cl in range(4):
                    ps = ps_out.tile([128, 512], F32, tag="po")
                    for h in range(HL):
                        nc.tensor.matmul(
                            ps[:],
                            oT[h][:, 2 * tt:2 * tt + 2, :],
                            wo_sb[:, h, cl * 512:(cl + 1) * 512],
                            start=(h == 0), stop=(h == HL - 1))
                    osb = scr.tile([128, 512], F32, tag="outsb", bufs=3)
                    nc.scalar.copy(osb[:], ps[:])
                    nc.sync.dma_start(
                        d_out[tt * 128:(tt + 1) * 128, cl * 512:(cl + 1) * 512],
                        osb[:])
    nc.compile()
    return nc


def _host_inputs(inputs):
    """Build the 8 per-core input maps from the full-problem inputs."""
    f32 = np.float32
    bf16 = ml_dtypes.bfloat16
    X = np.ascontiguousarray(np.asarray(inputs["hidden_states"], f32)[0])  # [T, HID]
    XT = np.ascontiguousarray(X.T).astype(bf16)

    tri_neg = np.triu(np.full((C, C), -1.0, f32))          # [j,i] -1 if j<=i
    tri_rev = np.tril(np.full((C, C), -1.0, f32), -1)      # -1 if j>i
    negc31 = np.zeros((C, C), f32); negc31[:32, :] = -1.0  # -1 if j<=31
    maskS = np.triu(np.full((C, C), SCALE, f32))
    maskJ = np.triu(np.ones((C, C), f32))
    negones = np.full((C, 128), -1.0, f32)
    ident = np.eye(128, dtype=f32)
    ones1 = np.ones((1, C), f32)

    Wo_full = np.asarray(inputs["Wo"], f32) * np.tile(
        np.asarray(inputs["norm_w"], f32), H)[:, None]

    in_maps = []
    for c in range(8):
        hsl = slice(c * HL * 128, (c + 1) * HL * 128)
        bsl = slice(c * HL, (c + 1) * HL)
        m = {
            "xt": XT,
            "wq": np.asarray(inputs["Wq"], f32)[:, hsl].astype(bf16),
            "wk": np.asarray(inputs["Wk"], f32)[:, hsl].astype(bf16),
            "wv": np.asarray(inputs["Wv"], f32)[:, hsl].astype(bf16),
            "ww": np.asarray(inputs["Ww"], f32)[:, hsl].astype(bf16),
            "wf1": np.asarray(inputs["Wf1"], f32).astype(bf16),
            "wg1": np.asarray(inputs["Wg1"], f32).astype(bf16),
            "wb": np.asarray(inputs["Wb"], f32)[:, bsl].astype(bf16),
            "wf2": np.ascontiguousarray(np.asarray(inputs["Wf2"], f32)[:, hsl]),
            "wg2": np.ascontiguousarray(np.asarray(inputs["Wg2"], f32)[:, hsl]),
            "bg2": np.ascontiguousarray(
                np.asarray(inputs["bg2"], f32)[None, hsl]),
            "wo": np.ascontiguousarray(Wo_full[hsl]).astype(bf16),
            "cq": np.ascontiguousarray(
                np.asarray(inputs["cq"], f32)[hsl].reshape(HL, 128, KW)
                .transpose(1, 0, 2)),
            "ck": np.ascontiguousarray(
                np.asarray(inputs["ck"], f32)[hsl].reshape(HL, 128, KW)
                .transpose(1, 0, 2)),
            "cv": np.ascontiguousarray(
                np.asarray(inputs["cv"], f32)[hsl].reshape(HL, 128, KW)
                .transpose(1, 0, 2)),
            "trineg": tri_neg, "trirev": tri_rev, "negc31": negc31,
            "masks": maskS, "maskj": maskJ.astype(np.uint8), "negones": negones,
            "ident": ident, "ones1": ones1,
        }
        in_maps.append(m)
    return in_maps


def kernel(_trace=False, **inputs):
    if "nc" not in _CACHE:
        _CACHE["nc"] = _build_nc()
    nc = _CACHE["nc"]
    in_maps = _host_inputs(inputs)
    res = run_bass_kernel_spmd(nc, in_maps, core_ids=list(range(8)),
                               trace=_trace)
    _CACHE["last_result"] = res
    out = np.zeros((T, HID), np.float32)
    for r in res.results:
        out += r["out"]
    return out.reshape(B, T, HID)



# revision 12
# speedup vs baseline: 1.6134x; 1.6134x over previous
"""GatedSlotAttention2 Trainium2 Bass kernel (v2).

Sharding: 2 heads per core x 8 cores (H=16). Each core runs the full
pipeline for its two heads and emits a partial Wo product; the host sums
the 8 bf16 partials in f32.

v2 redesign vs baseline:
- C=128 chunks (8 serial steps instead of 16), three-block rt matmul with
  per-block gate offsets for overflow safety.
- bf16 matmuls everywhere off the state-accumulation path (4x PE rate).
- Denominator-free softmax: RMSNorm is scale-invariant; the dropped den
  is folded into the RMSNorm eps term (eps*den^2) via the Ln bias.
- Activation-table discipline: silu/sigmoid/softplus/ln/exp phases are
  grouped so the scalar engine loads each table O(1) times.
- Everything off the 8-step state recurrence is batched outside the
  serial chain; copies spread across scalar/gpsimd engines.
"""
import numpy as np
import ml_dtypes

import concourse.bass as bass
import concourse.bacc as bacc_mod
import concourse.mybir as mybir
import concourse.tile as tile
from concourse.bass_utils import run_bass_kernel_spmd

F32 = mybir.dt.float32
BF16 = mybir.dt.bfloat16
AF = mybir.ActivationFunctionType
ALU = mybir.AluOpType
MS = bass.MemorySpace
AX = mybir.AxisListType

B, T, HID = 1, 1024, 2048
H, DK, DV, M, KW = 16, 128, 128, 128, 4
SCALE = DK ** -0.5
EPS = 1e-5
C = 128           # chunk length
NCH = T // C      # 8 chunks
NKT = HID // 128  # 16 contraction tiles
HL = 2            # heads per core
MID = 63

_CACHE = {}


def _build_nc():
    nc = bacc_mod.Bacc("TRN2")

    # ---------------- DRAM I/O ----------------
    d_xt = nc.dram_tensor("xt", [HID, T], BF16, kind="ExternalInput")
    d_wq = nc.dram_tensor("wq", [HID, HL * DK], BF16, kind="ExternalInput")
    d_wk = nc.dram_tensor("wk", [HID, HL * DK], BF16, kind="ExternalInput")
    d_wv = nc.dram_tensor("wv", [HID, HL * DV], BF16, kind="ExternalInput")
    d_ww = nc.dram_tensor("ww", [HID, HL * M], BF16, kind="ExternalInput")
    d_wf1 = nc.dram_tensor("wf1", [HID, DV], BF16, kind="ExternalInput")
    d_wg1 = nc.dram_tensor("wg1", [HID, DV], BF16, kind="ExternalInput")
    d_wb = nc.dram_tensor("wb", [HID, HL], BF16, kind="ExternalInput")
    d_wf2 = nc.dram_tensor("wf2", [DV, HL * M], BF16, kind="ExternalInput")
    d_wg2 = nc.dram_tensor("wg2", [DV, HL * DV], BF16, kind="ExternalInput")
    d_bg2 = nc.dram_tensor("bg2", [1, HL * DV], BF16, kind="ExternalInput")
    d_wo = nc.dram_tensor("wo", [HL * DV, HID], BF16, kind="ExternalInput")
    d_cq = nc.dram_tensor("cq", [128, HL, KW], F32, kind="ExternalInput")
    d_ck = nc.dram_tensor("ck", [128, HL, KW], F32, kind="ExternalInput")
    d_cv = nc.dram_tensor("cv", [128, HL, KW], F32, kind="ExternalInput")
    # constants
    d_trineg = nc.dram_tensor("trineg", [C, C], F32, kind="ExternalInput")   # -1 if j<=i
    d_nb1A = nc.dram_tensor("nb1a", [C, C], F32, kind="ExternalInput")       # piecewise offsets
    d_neg63 = nc.dram_tensor("neg63", [C, C], F32, kind="ExternalInput")     # -1 if j<=63
    d_negall = nc.dram_tensor("negall", [C, C], F32, kind="ExternalInput")   # all -1
    d_negcol = nc.dram_tensor("negcol", [C, 1], F32, kind="ExternalInput")   # col of -1
    d_onesrb = nc.dram_tensor("onesrb", [1, C], BF16, kind="ExternalInput")
    d_maskJ = nc.dram_tensor("maskj", [C, C], BF16, kind="ExternalInput")    # 1 if j<=i
    d_identb = nc.dram_tensor("identb", [128, 128], BF16, kind="ExternalInput")
    d_identf = nc.dram_tensor("identf", [128, 128], F32, kind="ExternalInput")

    d_out = nc.dram_tensor("out", [T, HID], BF16, kind="ExternalOutput")

    with tile.TileContext(nc) as tc:
        with (
            tc.tile_pool(name="persist", bufs=1) as pp,
            tc.tile_pool(name="wstage", bufs=2) as wsp,
            tc.tile_pool(name="cacc", bufs=2) as ccp,
            tc.tile_pool(name="xpad", bufs=2) as xpp,
            tc.tile_pool(name="scrA", bufs=2) as scA,      # [128,256] f32 rotators
            tc.tile_pool(name="scrB", bufs=2) as scB,      # [128,256] bf16 exps
            tc.tile_pool(name="scrC", bufs=3) as scC,      # [128,128] bf16 per-head
            tc.tile_pool(name="scrD", bufs=2) as scD,      # f32 [128,128] per-head
            tc.tile_pool(name="states", bufs=2) as stp,
            tc.tile_pool(name="tiny", bufs=3) as tnp,
            tc.tile_pool(name="ps_big", bufs=2, space=MS.PSUM) as ps_big,
            tc.tile_pool(name="ps_cum", bufs=2, space=MS.PSUM) as ps_cum,
            tc.tile_pool(name="ps_sc", bufs=2, space=MS.PSUM) as ps_sc,
            tc.tile_pool(name="ps_tr", bufs=2, space=MS.PSUM) as ps_tr,
        ):
            # ---------- constants to SBUF ----------
            def load_const(dram, shape, dtype=F32):
                t = pp.tile(shape, dtype, tag=dram.name + "_sb")
                nc.sync.dma_start(t[:], dram[:])
                return t

            c_trineg = load_const(d_trineg, [C, C])
            c_nb1A = load_const(d_nb1A, [C, C])
            c_neg63 = load_const(d_neg63, [C, C])
            c_negall = load_const(d_negall, [C, C])
            c_negcol = load_const(d_negcol, [C, 1])
            c_onesrb = load_const(d_onesrb, [1, C], BF16)
            c_maskJ = load_const(d_maskJ, [C, C], BF16)
            c_identb = load_const(d_identb, [128, 128], BF16)
            c_identf = load_const(d_identf, [128, 128], F32)
            c_wf2 = load_const(d_wf2, [DV, HL * M], BF16)
            c_wg2 = load_const(d_wg2, [DV, HL * DV], BF16)
            c_bg2 = load_const(d_bg2, [1, HL * DV], BF16)
            c_cq = load_const(d_cq, [128, HL, KW])
            c_ck = load_const(d_ck, [128, HL, KW])
            c_cv = load_const(d_cv, [128, HL, KW])
            c_eps6 = pp.tile([C, 1], F32, tag="c_eps6")
            nc.vector.memset(c_eps6[:], 1e-6)

            # ---------- X^T + Wo ----------
            xt_sb = pp.tile([128, NKT, T], BF16, tag="xt_sb")
            xtr = d_xt.rearrange("(k p) t -> k p t", p=128)
            for kt in range(NKT):
                nc.sync.dma_start(xt_sb[:, kt, :], xtr[kt])

            wo_sb = pp.tile([128, HL, HID], BF16, tag="wo_sb")
            wor = d_wo.rearrange("(h p) o -> h p o", p=128)
            for h in range(HL):
                nc.sync.dma_start(wo_sb[:, h, :], wor[h])

            # ---------- P1: projections + conv (silu deferred) ----------
            qT = pp.tile([128, HL, T], BF16, tag="qT")
            kT = pp.tile([128, HL, T], BF16, tag="kT")
            vTc = pp.tile([128, HL, T], BF16, tag="vTc")
            wTc = pp.tile([128, HL, T], BF16, tag="wTc")

            silu_jobs = []  # (cacc_tile, out_ap) for the grouped Silu phase

            def project_conv(d_w, c_cw, out_tile):
                w_sb = wsp.tile([128, NKT, HL * 128], BF16, tag="w_load")
                wr = d_w.rearrange("(k p) c -> k p c", p=128)
                for kt in range(NKT):
                    nc.sync.dma_start(w_sb[:, kt, :], wr[kt])
                for ct in range(HL):
                    xpad = xpp.tile([128, T + KW - 1], BF16, tag="xpad")
                    nc.vector.memset(xpad[:, 0:KW - 1], 0.0)
                    for tt in range(2):
                        ps = ps_big.tile([128, 512], F32, tag="pp")
                        for kt in range(NKT):
                            nc.tensor.matmul(
                                ps[:],
                                w_sb[:, kt, ct * 128:(ct + 1) * 128],
                                xt_sb[:, kt, tt * 512:(tt + 1) * 512],
                                start=(kt == 0), stop=(kt == NKT - 1),
                            )
                        dst = xpad[:, KW - 1 + tt * 512: KW - 1 + (tt + 1) * 512]
                        if tt == 0:
                            nc.scalar.copy(dst, ps[:])
                        else:
                            nc.vector.tensor_copy(dst, ps[:])
                    cacc = ccp.tile([128, T], F32, tag="cacc")
                    # conv taps: DVE half uses fused stt; Pool half mul+add
                    hs = slice(0, 512)
                    nc.vector.tensor_scalar_mul(
                        cacc[:, hs], xpad[:, 0:512], c_cw[:, ct, 0:1])
                    for i in range(1, KW):
                        nc.vector.scalar_tensor_tensor(
                            cacc[:, hs], xpad[:, i:i + 512],
                            c_cw[:, ct, i:i + 1], cacc[:, hs],
                            op0=ALU.mult, op1=ALU.add)
                    hs = slice(512, 1024)
                    ctmp = ccp.tile([128, 512], F32, tag="ctmp")
                    nc.gpsimd.tensor_scalar_mul(
                        cacc[:, hs], xpad[:, 512:1024], c_cw[:, ct, 0:1])
                    for i in range(1, KW):
                        nc.gpsimd.tensor_scalar_mul(
                            ctmp[:], xpad[:, 512 + i:512 + i + 512],
                            c_cw[:, ct, i:i + 1])
                        nc.gpsimd.tensor_add(cacc[:, hs], cacc[:, hs], ctmp[:])
                    silu_jobs.append((cacc, out_tile[:, ct, :]))

            project_conv(d_wq, c_cq, qT)
            project_conv(d_wk, c_ck, kT)
            project_conv(d_wv, c_cv, vTc)
            project_conv(d_ww, c_cv, wTc)

            # grouped Silu phase (one act table load)
            for cacc, out_ap in silu_jobs:
                nc.scalar.activation(out_ap, cacc[:], AF.Silu)

            # ---------- f1T, g1T (no conv) ----------
            def proj128T(d_w, tag):
                out = pp.tile([128, T], BF16, tag=tag)
                w_sb = wsp.tile([128, NKT, 128], BF16, tag="w_load")
                wr = d_w.rearrange("(k p) c -> k p c", p=128)
                for kt in range(NKT):
                    nc.sync.dma_start(w_sb[:, kt, :], wr[kt])
                for tt in range(2):
                    ps = ps_big.tile([128, 512], F32, tag="pp")
                    for kt in range(NKT):
                        nc.tensor.matmul(
                            ps[:], w_sb[:, kt, :],
                            xt_sb[:, kt, tt * 512:(tt + 1) * 512],
                            start=(kt == 0), stop=(kt == NKT - 1))
                    if tt == 0:
                        nc.scalar.copy(out[:, tt * 512:(tt + 1) * 512], ps[:])
                    else:
                        nc.vector.tensor_copy(out[:, tt * 512:(tt + 1) * 512], ps[:])
                return out

            f1T = proj128T(d_wf1, "f1T")
            g1T = proj128T(d_wg1, "g1T")

            # ---------- beta ----------
            betaT = pp.tile([HL, T], F32, tag="betaT")
            wb_sb = wsp.tile([128, NKT, HL], BF16, tag="wb_load")
            wbr = d_wb.rearrange("(k p) c -> k p c", p=128)
            for kt in range(NKT):
                nc.sync.dma_start(wb_sb[:, kt, :], wbr[kt])
            beta_ps = []
            for tt in range(2):
                ps = ps_big.tile([128, 512], F32, tag="pp")
                for kt in range(NKT):
                    nc.tensor.matmul(
                        ps[0:HL, :], wb_sb[:, kt, :],
                        xt_sb[:, kt, tt * 512:(tt + 1) * 512],
                        start=(kt == 0), stop=(kt == NKT - 1))
                beta_ps.append(ps)

            # ---------- gate (matmul now; sigmoid grouped below) ----------
            gate_sb = pp.tile([128, NCH, HL * DV], BF16, tag="gate_sb")
            gate_ps = []
            for n in range(NCH):
                ps = ps_cum.tile([128, HL * DV], F32, tag="cum")
                nc.tensor.matmul(ps[:], g1T[:, n * C:(n + 1) * C], c_wg2[:],
                                 start=True, stop=False)
                nc.tensor.matmul(ps[:], c_onesrb[:], c_bg2[:],
                                 start=False, stop=True)
                gate_ps.append(ps)
            # sigmoid phase: beta then gate (one table load)
            for tt in range(2):
                nc.scalar.activation(betaT[:, tt * 512:(tt + 1) * 512],
                                     beta_ps[tt][0:HL, :], AF.Sigmoid)
            for n in range(NCH):
                nc.scalar.activation(gate_sb[:, n, :], gate_ps[n][:], AF.Sigmoid)

            # ---------- gpos = ln(1 + exp(-s)), exp/ln phases grouped ----------
            gpos_sb = pp.tile([128, NCH, HL * M], F32, tag="gpos_sb")
            gpe_sb = pp.tile([128, NCH, HL * M], F32, tag="gpe_sb")
            for n in range(NCH):
                ps = ps_cum.tile([128, HL * M], F32, tag="cum")
                nc.tensor.matmul(ps[:], f1T[:, n * C:(n + 1) * C], c_wf2[:],
                                 start=True, stop=True)
                nc.scalar.activation(gpe_sb[:, n, :], ps[:], AF.Exp,
                                     scale=-1.0)
            for n in range(NCH):
                nc.scalar.activation(gpos_sb[:, n, :], gpe_sb[:, n, :],
                                     AF.Ln, bias=1.0)

            # ---------- beta transpose -> [t, HL] ----------
            beta_t = pp.tile([128, NCH, HL], F32, tag="beta_t")
            for n in range(NCH):
                ps = ps_big.tile([128, HL], F32, tag="pp")
                nc.tensor.matmul(ps[:], betaT[:, n * C:(n + 1) * C],
                                 c_identf[0:HL, 0:HL], is_transpose=True)
                nc.scalar.copy(beta_t[:, n, :], ps[:])

            # ---------- W transpose + l2norm + beta -> bw ----------
            bw2 = pp.tile([128, NCH, HL * M], BF16, tag="bw2")
            rs_list = []
            for n in range(NCH):
                for h in range(HL):
                    psw = ps_tr.tile([128, 128], BF16, tag="trb")
                    nc.tensor.transpose(psw[:], wTc[:, h, n * C:(n + 1) * C],
                                        c_identb[:])
                    Wc = scC.tile([128, 128], BF16, tag="Wc")
                    nc.vector.tensor_copy(Wc[:], psw[:])
                    junk = scC.tile([128, 128], BF16, tag="junkw")
                    ssq = tnp.tile([128, 1], F32, tag="ssq")
                    nc.vector.scalar_tensor_tensor(
                        junk[:], Wc[:], 1.0, Wc[:],
                        op0=ALU.mult, op1=ALU.mult, accum_out=ssq[:])
                    rs_list.append((n, h, Wc, ssq))
            # grouped Ln then Exp then muls
            lnr_all = pp.tile([128, NCH, HL], F32, tag="lnr_all")
            for n, h, Wc, ssq in rs_list:
                nc.scalar.activation(lnr_all[:, n, h:h + 1], ssq[:], AF.Ln,
                                     bias=c_eps6[:])
            for i, (n, h, Wc, ssq) in enumerate(rs_list):
                rst = tnp.tile([128, 1], F32, tag="rst")
                nc.scalar.activation(rst[:], lnr_all[:, n, h:h + 1], AF.Exp,
                                     scale=-0.5)
                rsb = tnp.tile([128, 1], F32, tag="rsb")
                nc.vector.tensor_mul(rsb[:], rst[:], beta_t[:, n, h:h + 1])
                nc.vector.tensor_scalar_mul(
                    bw2[:, n, h * M:(h + 1) * M], Wc[:], rsb[:])

            # ---------- states ----------
            Sk_cur = [None, None]
            Sv_cur = [None, None]
            Skb = [None, None]
            Svb = [None, None]

            den_sb = pp.tile([128, NCH, HL], F32, tag="den_sb")
            o_sb = pp.tile([128, NCH, HL, DV], BF16, tag="o_sb")
            oT_sb = pp.tile([128, HL, T], BF16, tag="oT_sb")

            # ---------- main chunk loop ----------
            for n in range(NCH):
                t0 = n * C
                tsl = slice(t0, t0 + C)
                # --- gate cumsums (both heads at once) ---
                cum_ps = ps_cum.tile([128, HL * M], F32, tag="cum")
                nc.tensor.matmul(cum_ps[:], c_trineg[:], gpos_sb[:, n, :],
                                 start=True, stop=True)
                Gc = scA.tile([128, HL * M], F32, tag="Gc")
                nc.scalar.copy(Gc[:], cum_ps[:])
                b1A_ps = ps_cum.tile([128, HL * M], F32, tag="cum")
                nc.tensor.matmul(b1A_ps[:], c_nb1A[:], gpos_sb[:, n, :],
                                 start=True, stop=True)
                b63_ps = ps_cum.tile([128, HL * M], F32, tag="cum")
                nc.tensor.matmul(b63_ps[:], c_neg63[:], gpos_sb[:, n, :],
                                 start=True, stop=True)
                blast_ps = ps_cum.tile([128, HL * M], F32, tag="cum")
                nc.tensor.matmul(blast_ps[:], c_negall[:], gpos_sb[:, n, :],
                                 start=True, stop=True)
                GcpA = scA.tile([128, HL * M], F32, tag="GcpA")
                nc.vector.tensor_sub(GcpA[:], Gc[:], b1A_ps[:])
                Gcp63 = scA.tile([128, HL * M], F32, tag="Gcp63")
                nc.vector.tensor_sub(Gcp63[:], Gc[:], b63_ps[:])
                grev = scA.tile([128, HL * M], F32, tag="grev")
                nc.vector.tensor_sub(grev[:], blast_ps[:], Gc[:])
                # LamCc per head: exp of per-slot total decay as a column
                lcc = [None, None]
                for h in range(HL):
                    hs = slice(h * M, (h + 1) * M)
                    lcc_ps = ps_cum.tile([128, 1], F32, tag="cum")
                    nc.tensor.matmul(lcc_ps[0:M, :], gpos_sb[:, n, hs],
                                     c_negcol[:], start=True, stop=True)
                    lcv = tnp.tile([M, 1], F32, tag="lcc_sb")
                    nc.scalar.activation(lcv[:], lcc_ps[0:M, :], AF.Exp)
                    lcc[h] = lcv
                # --- exps (all on exp table) ---
                Lam = scB.tile([128, HL * M], BF16, tag="Lam")
                nc.scalar.activation(Lam[:], Gc[:], AF.Exp)
                EposA = scB.tile([128, HL * M], BF16, tag="EposA")
                nc.scalar.activation(EposA[:], GcpA[:], AF.Exp)
                Epos63 = scB.tile([128, HL * M], BF16, tag="Epos63")
                nc.scalar.activation(Epos63[:], Gcp63[:], AF.Exp)
                EnegAe = scB.tile([128, HL * M], BF16, tag="EnegAe")
                nc.scalar.activation(EnegAe[:], GcpA[:], AF.Exp, scale=-1.0)
                Eneg63e = scB.tile([128, HL * M], BF16, tag="Eneg63e")
                nc.scalar.activation(Eneg63e[:], Gcp63[:], AF.Exp, scale=-1.0)
                Ereve = scB.tile([128, HL * M], BF16, tag="Ereve")
                nc.scalar.activation(Ereve[:], grev[:], AF.Exp)
                LamCb = scA.tile([128, HL * M], F32, tag="LamCb")
                nc.scalar.activation(LamCb[:], blast_ps[:], AF.Exp)
                # --- bw muls ---
                EnegA = scB.tile([128, HL * M], BF16, tag="EnegA")
                nc.vector.tensor_mul(EnegA[:], EnegAe[:], bw2[:, n, :])
                Eneg63 = scB.tile([128, HL * M], BF16, tag="Eneg63")
                nc.vector.tensor_mul(Eneg63[:], Eneg63e[:], bw2[:, n, :])
                Kdec = scB.tile([128, HL * M], BF16, tag="Kdec")
                nc.gpsimd.tensor_mul(Kdec[:], Ereve[:], bw2[:, n, :])

                for h in range(HL):
                    hs = slice(h * M, (h + 1) * M)
                    hv = slice(h * DV, (h + 1) * DV)
                    # --- K/V transposes for this chunk ---
                    psk = ps_tr.tile([128, 128], BF16, tag="trb")
                    nc.tensor.transpose(psk[:], kT[:, h, tsl], c_identb[:])
                    Kc = scC.tile([128, 128], BF16, tag="Kc")
                    nc.scalar.copy(Kc[:], psk[:])
                    psv = ps_tr.tile([128, 128], BF16, tag="trb")
                    nc.tensor.transpose(psv[:], vTc[:, h, tsl], c_identb[:])
                    Vc = scC.tile([128, 128], BF16, tag="Vc")
                    nc.scalar.copy(Vc[:], psv[:])
                    # --- Eneg transposes ---
                    pse = ps_tr.tile([128, 128], BF16, tag="trb")
                    nc.tensor.transpose(pse[:], EnegA[:, hs], c_identb[:])
                    EnegAT = scC.tile([128, 128], BF16, tag="EnegAT")
                    nc.vector.tensor_copy(EnegAT[:], pse[:])
                    ps6 = ps_tr.tile([128, 64], BF16, tag="trb")
                    nc.tensor.transpose(ps6[:], Eneg63[0:64, hs],
                                        c_identb[0:64, 0:64])
                    En63Tu = scC.tile([128, 64], BF16, tag="En63Tu")
                    nc.scalar.copy(En63Tu[:], ps6[:])
                    # --- pt + mask ---
                    pt_ps = ps_sc.tile([128, 128], F32, tag="sc")
                    nc.tensor.matmul(pt_ps[:], kT[:, h, tsl], qT[:, h, tsl],
                                     start=True, stop=True)
                    Ptm = scC.tile([128, 128], BF16, tag="Ptm")
                    nc.vector.scalar_tensor_tensor(
                        Ptm[:], pt_ps[:], SCALE, c_maskJ[:],
                        op0=ALU.mult, op1=ALU.mult)
                    # --- intra + s2 ---
                    intra_ps = ps_sc.tile([128, 128], F32, tag="sc")
                    nc.tensor.matmul(intra_ps[:], Ptm[:], Eneg63[:, hs],
                                     start=True, stop=True)
                    s2 = scD.tile([128, 128], F32, tag="s2")
                    nc.vector.tensor_mul(s2[:], intra_ps[:], Epos63[:, hs])
                    # --- scores ---
                    if n == 0:
                        sS = s2
                    else:
                        qs_ps = ps_sc.tile([128, 128], F32, tag="sc")
                        nc.tensor.matmul(qs_ps[:], qT[:, h, tsl], Skb[h][:],
                                         start=True, stop=True)
                        s1 = scD.tile([128, 128], F32, tag="s1")
                        nc.vector.scalar_tensor_tensor(
                            s1[:], qs_ps[:], SCALE, Lam[:, hs],
                            op0=ALU.mult, op1=ALU.mult)
                        sS = scD.tile([128, 128], F32, tag="sS")
                        nc.vector.tensor_add(sS[:], s1[:], s2[:])
                    nmx = tnp.tile([128, 1], F32, tag="nmx")
                    nc.vector.tensor_reduce(nmx[:], sS[:], AX.X, ALU.max,
                                            negate=True)
                    pexp = scC.tile([128, 128], BF16, tag="pexp")
                    nc.scalar.activation(pexp[:], sS[:], AF.Exp, bias=nmx[:],
                                         accum_out=den_sb[:, n, h:h + 1])
                    # --- attention weights ---
                    aL = scC.tile([128, 128], BF16, tag="aL")
                    nc.vector.tensor_mul(aL[:], pexp[:], Lam[:, hs])
                    aEA = scC.tile([128, 128], BF16, tag="aEA")
                    nc.vector.tensor_mul(aEA[:], pexp[:], EposA[:, hs])
                    aE63u = scC.tile([64, 128], BF16, tag="aE63u")
                    nc.gpsimd.tensor_mul(aE63u[:], pexp[64:128, :],
                                         Epos63[64:128, hs])
                    # transposes
                    psl = ps_tr.tile([128, 128], BF16, tag="trb")
                    nc.tensor.transpose(psl[:], aL[:], c_identb[:])
                    aLT = scC.tile([128, 128], BF16, tag="aLT")
                    nc.scalar.copy(aLT[:], psl[:])
                    psa = ps_tr.tile([128, 128], BF16, tag="trb")
                    nc.tensor.transpose(psa[:], aEA[:], c_identb[:])
                    aEAT = scC.tile([128, 128], BF16, tag="aEAT")
                    nc.vector.tensor_copy(aEAT[:], psa[:])
                    ps63 = ps_tr.tile([128, 64], BF16, tag="trb")
                    nc.tensor.transpose(ps63[:], aE63u[:],
                                        c_identb[0:64, 0:64])
                    aE63uT = scC.tile([128, 64], BF16, tag="aE63uT")
                    nc.scalar.copy(aE63uT[:], ps63[:])
                    # --- rt blocks ---
                    rt_ps = ps_sc.tile([128, 128], F32, tag="sc")
                    nc.tensor.matmul(rt_ps[0:64, 0:64], EnegAT[:, 0:64],
                                     aEAT[:, 0:64], start=True, stop=True)
                    nc.tensor.matmul(rt_ps[64:128, 64:128], EnegAT[:, 64:128],
                                     aEAT[:, 64:128], start=True, stop=True)
                    nc.tensor.matmul(rt_ps[0:64, 64:128], En63Tu[:],
                                     aE63uT[:], start=True, stop=True)
                    Rmt = scC.tile([128, 128], BF16, tag="Rmt")
                    nc.vector.memset(Rmt[64:128, 0:64], 0.0)
                    nc.vector.tensor_mul(Rmt[0:64, :], rt_ps[0:64, :],
                                         c_maskJ[0:64, :])
                    nc.vector.tensor_mul(Rmt[64:128, 64:128],
                                         rt_ps[64:128, 64:128],
                                         c_maskJ[64:128, 64:128])
                    # --- output ---
                    o_ps = ps_sc.tile([128, 128], F32, tag="sc")
                    if n == 0:
                        nc.tensor.matmul(o_ps[:], Rmt[:], Vc[:],
                                         start=True, stop=True)
                    else:
                        nc.tensor.matmul(o_ps[:], aLT[:], Svb[h][:],
                                         start=True, stop=False)
                        nc.tensor.matmul(o_ps[:], Rmt[:], Vc[:],
                                         start=False, stop=True)
                    nc.scalar.copy(o_sb[:, n, h, :], o_ps[:])
                    # --- state update (skip at last chunk) ---
                    if n < NCH - 1:
                        skk_ps = ps_sc.tile([128, 128], F32, tag="sc")
                        nc.tensor.matmul(skk_ps[:], Kc[:], Kdec[:, hs],
                                         start=True, stop=True)
                        svk_ps = ps_sc.tile([128, 128], F32, tag="sc")
                        nc.tensor.matmul(svk_ps[:], Kdec[:, hs], Vc[:],
                                         start=True, stop=True)
                        Sk_new = stp.tile([128, 128], F32, tag=f"Sk{h}")
                        Sv_new = stp.tile([128, 128], F32, tag=f"Sv{h}")
                        if n == 0:
                            nc.vector.tensor_copy(Sk_new[:], skk_ps[:])
                            nc.vector.tensor_copy(Sv_new[:], svk_ps[:])
                        else:
                            skt = scD.tile([128, 128], F32, tag="skt")
                            nc.vector.tensor_mul(skt[:], Sk_cur[h][:],
                                                 LamCb[:, hs])
                            nc.vector.tensor_add(Sk_new[:], skt[:], skk_ps[:])
                            nc.vector.scalar_tensor_tensor(
                                Sv_new[:], Sv_cur[h][:], lcc[h][:], svk_ps[:],
                                op0=ALU.mult, op1=ALU.add)
                        Sk_cur[h] = Sk_new
                        Sv_cur[h] = Sv_new
                        Skb_new = stp.tile([128, 128], BF16, tag=f"Skb{h}")
                        nc.gpsimd.tensor_copy(Skb_new[:], Sk_new[:])
                        Svb_new = stp.tile([128, 128], BF16, tag=f"Svb{h}")
                        nc.gpsimd.tensor_copy(Svb_new[:], Sv_new[:])
                        Skb[h] = Skb_new
                        Svb[h] = Svb_new

            # ---------- P6: epilogue (RMSNorm w/ den^2, gate, transpose) ----------
            d2_all = pp.tile([128, NCH, HL], F32, tag="d2_all")
            orm_all = pp.tile([128, NCH, HL], F32, tag="orm_all")
            ep = []
            for n in range(NCH):
                for h in range(HL):
                    osum = tnp.tile([128, 1], F32, tag="osum")
                    junk2 = scC.tile([128, 128], BF16, tag="junko")
                    nc.vector.scalar_tensor_tensor(
                        junk2[:], o_sb[:, n, h, :], 1.0, o_sb[:, n, h, :],
                        op0=ALU.mult, op1=ALU.mult, accum_out=osum[:])
                    nc.scalar.activation(d2_all[:, n, h:h + 1],
                                         den_sb[:, n, h:h + 1],
                                         AF.Square, scale=EPS ** 0.5)
                    ep.append((n, h, osum))
            for n, h, osum in ep:
                nc.scalar.activation(orm_all[:, n, h:h + 1], osum[:], AF.Ln,
                                     scale=1.0 / DV, bias=d2_all[:, n, h:h + 1])
            for n, h, osum in ep:
                orr = tnp.tile([128, 1], F32, tag="orr")
                nc.scalar.activation(orr[:], orm_all[:, n, h:h + 1], AF.Exp,
                                     scale=-0.5)
                of = scC.tile([128, 128], BF16, tag="of")
                nc.vector.scalar_tensor_tensor(
                    of[:], o_sb[:, n, h, :], orr[:],
                    gate_sb[:, n, h * DV:(h + 1) * DV],
                    op0=ALU.mult, op1=ALU.mult)
                pso = ps_tr.tile([128, 128], BF16, tag="trb")
                nc.tensor.transpose(pso[:], of[:], c_identb[:])
                nc.scalar.copy(oT_sb[:, h, n * C:(n + 1) * C], pso[:])

            # ---------- Wo partial + DMA out ----------
            for tt in range(NCH):
                for cl in range(4):
                    ps = ps_big.tile([128, 512], F32, tag="pp")
                    for h in range(HL):
                        nc.tensor.matmul(
                            ps[:], oT_sb[:, h, tt * C:(tt + 1) * C],
                            wo_sb[:, h, cl * 512:(cl + 1) * 512],
                            start=(h == 0), stop=(h == HL - 1))
                    osb = scC.tile([128, 512], BF16, tag="outsb")
                    if cl % 2 == 0:
                        nc.scalar.copy(osb[:], ps[:])
                    else:
                        nc.vector.tensor_copy(osb[:], ps[:])
                    nc.sync.dma_start(
                        d_out[tt * 128:(tt + 1) * 128, cl * 512:(cl + 1) * 512],
                        osb[:])
    nc.compile()
    return nc


def _host_inputs(inputs):
    f32 = np.float32
    bf16 = ml_dtypes.bfloat16
    X = np.ascontiguousarray(np.asarray(inputs["hidden_states"], f32)[0])
    XT = np.ascontiguousarray(X.T).astype(bf16)

    trineg = np.triu(np.full((C, C), -1.0, f32))
    nb1a = np.zeros((C, C), f32)     # [j, p]: -1 if (p<64, j<=31) or (p>=64, j<=95)
    nb1a[0:32, 0:64] = -1.0
    nb1a[0:96, 64:128] = -1.0
    neg63 = np.zeros((C, C), f32)
    neg63[0:64, :] = -1.0
    negall = np.full((C, C), -1.0, f32)
    negcol = np.full((C, 1), -1.0, f32)
    onesr = np.ones((1, C), f32)
    maskJ = np.triu(np.ones((C, C), f32))
    ident = np.eye(128, dtype=f32)

    Wo_full = np.asarray(inputs["Wo"], f32) * np.tile(
        np.asarray(inputs["norm_w"], f32), H)[:, None]

    in_maps = []
    for c in range(8):
        hsl = slice(c * HL * 128, (c + 1) * HL * 128)
        bsl = slice(c * HL, (c + 1) * HL)
        m = {
            "xt": XT,
            "wq": np.asarray(inputs["Wq"], f32)[:, hsl].astype(bf16),
            "wk": np.asarray(inputs["Wk"], f32)[:, hsl].astype(bf16),
            "wv": np.asarray(inputs["Wv"], f32)[:, hsl].astype(bf16),
            "ww": np.asarray(inputs["Ww"], f32)[:, hsl].astype(bf16),
            "wf1": np.asarray(inputs["Wf1"], f32).astype(bf16),
            "wg1": np.asarray(inputs["Wg1"], f32).astype(bf16),
            "wb": np.asarray(inputs["Wb"], f32)[:, bsl].astype(bf16),
            "wf2": np.ascontiguousarray(
                np.asarray(inputs["Wf2"], f32)[:, hsl]).astype(bf16),
            "wg2": np.ascontiguousarray(
                np.asarray(inputs["Wg2"], f32)[:, hsl]).astype(bf16),
            "bg2": np.ascontiguousarray(
                np.asarray(inputs["bg2"], f32)[None, hsl]).astype(bf16),
            "wo": np.ascontiguousarray(Wo_full[hsl]).astype(bf16),
            "cq": np.ascontiguousarray(
                np.asarray(inputs["cq"], f32)[hsl].reshape(HL, 128, KW)
                .transpose(1, 0, 2)),
            "ck": np.ascontiguousarray(
                np.asarray(inputs["ck"], f32)[hsl].reshape(HL, 128, KW)
                .transpose(1, 0, 2)),
            "cv": np.ascontiguousarray(
                np.asarray(inputs["cv"], f32)[hsl].reshape(HL, 128, KW)
                .transpose(1, 0, 2)),
            "trineg": trineg, "nb1a": nb1a, "neg63": neg63, "negall": negall,
            "negcol": negcol,
            "onesrb": onesr.astype(bf16), "maskj": maskJ.astype(bf16),
            "identb": ident.astype(bf16), "identf": ident,
        }
        in_maps.append(m)
    return in_maps


def kernel(_trace=False, **inputs):
    if "nc" not in _CACHE:
        _CACHE["nc"] = _build_nc()
    nc = _CACHE["nc"]
    in_maps = _host_inputs(inputs)
    res = run_bass_kernel_spmd(nc, in_maps, core_ids=list(range(8)),
                               trace=_trace)
    _CACHE["last_result"] = res
    out = np.zeros((T, HID), np.float32)
    for r in res.results:
        out += np.asarray(r["out"], np.float32)
    return out.reshape(B, T, HID)


# revision 15
# speedup vs baseline: 2.9663x; 1.8385x over previous
"""GatedSlotAttention2 Trainium2 Bass kernel (v2).

Sharding: 2 heads per core x 8 cores (H=16). Each core runs the full
pipeline for its two heads and emits a partial Wo product; the host sums
the 8 bf16 partials in f32.

v2 redesign vs baseline:
- C=128 chunks (8 serial steps instead of 16), three-block rt matmul with
  per-block gate offsets for overflow safety.
- bf16 matmuls everywhere off the state-accumulation path (4x PE rate).
- Denominator-free softmax: RMSNorm is scale-invariant; the dropped den
  is folded into the RMSNorm eps term (eps*den^2) via the Ln bias.
- Activation-table discipline: silu/sigmoid/softplus/ln/exp phases are
  grouped so the scalar engine loads each table O(1) times.
- Everything off the 8-step state recurrence is batched outside the
  serial chain; copies spread across scalar/gpsimd engines.
"""
import numpy as np
import ml_dtypes

import concourse.bass as bass
import concourse.bacc as bacc_mod
import concourse.mybir as mybir
import concourse.tile as tile
from concourse.bass_utils import run_bass_kernel_spmd

F32 = mybir.dt.float32
BF16 = mybir.dt.bfloat16
AF = mybir.ActivationFunctionType
ALU = mybir.AluOpType
MS = bass.MemorySpace
AX = mybir.AxisListType

B, T, HID = 1, 1024, 2048
H, DK, DV, M, KW = 16, 128, 128, 128, 4
SCALE = DK ** -0.5
EPS = 1e-5
C = 128           # chunk length
NCH = T // C      # 8 chunks
NKT = HID // 128  # 16 contraction tiles
HL = 2            # heads per core
MID = 63

_CACHE = {}


def _build_nc():
    nc = bacc_mod.Bacc("TRN2")

    # ---------------- DRAM I/O ----------------
    d_xt = nc.dram_tensor("xt", [HID, T], BF16, kind="ExternalInput")
    d_wq = nc.dram_tensor("wq", [HID, HL * DK], BF16, kind="ExternalInput")
    d_wk = nc.dram_tensor("wk", [HID, HL * DK], BF16, kind="ExternalInput")
    d_wv = nc.dram_tensor("wv", [HID, HL * DV], BF16, kind="ExternalInput")
    d_ww = nc.dram_tensor("ww", [HID, HL * M], BF16, kind="ExternalInput")
    d_wf1 = nc.dram_tensor("wf1", [HID, DV], BF16, kind="ExternalInput")
    d_wg1 = nc.dram_tensor("wg1", [HID, DV], BF16, kind="ExternalInput")
    d_wb = nc.dram_tensor("wb", [HID, HL], BF16, kind="ExternalInput")
    d_wf2 = nc.dram_tensor("wf2", [DV, HL * M], BF16, kind="ExternalInput")
    d_wg2 = nc.dram_tensor("wg2", [DV, HL * DV], BF16, kind="ExternalInput")
    d_bg2 = nc.dram_tensor("bg2", [1, HL * DV], BF16, kind="ExternalInput")
    d_wo = nc.dram_tensor("wo", [HL * DV, HID], BF16, kind="ExternalInput")
    # conv weights as diagonal matrices: [proj(4) x ct(2) x tap(4)] of 128x128
    d_cdiag = nc.dram_tensor("cdiag", [128, 32, 128], BF16, kind="ExternalInput")
    # constants
    d_trineg = nc.dram_tensor("trineg", [C, C], F32, kind="ExternalInput")   # -1 if j<=i
    d_nb1A = nc.dram_tensor("nb1a", [C, C], F32, kind="ExternalInput")       # piecewise offsets
    d_neg63 = nc.dram_tensor("neg63", [C, C], F32, kind="ExternalInput")     # -1 if j<=63
    d_negall = nc.dram_tensor("negall", [C, C], F32, kind="ExternalInput")   # all -1
    d_negcol = nc.dram_tensor("negcol", [C, 1], F32, kind="ExternalInput")   # col of -1
    d_onesrb = nc.dram_tensor("onesrb", [1, C], BF16, kind="ExternalInput")
    d_maskJ = nc.dram_tensor("maskj", [C, C], BF16, kind="ExternalInput")    # 1 if j<=i
    d_identb = nc.dram_tensor("identb", [128, 128], BF16, kind="ExternalInput")
    d_identf = nc.dram_tensor("identf", [128, 128], F32, kind="ExternalInput")

    d_out = nc.dram_tensor("out", [T, HID], BF16, kind="ExternalOutput")

    with tile.TileContext(nc) as tc:
        with (
            tc.tile_pool(name="persist", bufs=1) as pp,
            tc.tile_pool(name="wstage", bufs=2) as wsp,
            tc.tile_pool(name="cacc", bufs=2) as ccp,
            tc.tile_pool(name="xpad", bufs=2) as xpp,
            tc.tile_pool(name="scrA", bufs=2) as scA,      # [128,256] f32 rotators
            tc.tile_pool(name="scrB", bufs=2) as scB,      # [128,256] bf16 exps
            tc.tile_pool(name="scrC", bufs=3) as scC,      # [128,128] bf16 per-head
            tc.tile_pool(name="scrD", bufs=2) as scD,      # f32 [128,128] per-head
            tc.tile_pool(name="states", bufs=2) as stp,
            tc.tile_pool(name="tiny", bufs=3) as tnp,
            tc.tile_pool(name="ps_big", bufs=2, space=MS.PSUM) as ps_big,
            tc.tile_pool(name="ps_cum", bufs=2, space=MS.PSUM) as ps_cum,
            tc.tile_pool(name="ps_sc", bufs=2, space=MS.PSUM) as ps_sc,
            tc.tile_pool(name="ps_tr", bufs=2, space=MS.PSUM) as ps_tr,
        ):
            # ---------- constants to SBUF ----------
            def load_const(dram, shape, dtype=F32):
                t = pp.tile(shape, dtype, tag=dram.name + "_sb")
                nc.sync.dma_start(t[:], dram[:])
                return t

            c_trineg = load_const(d_trineg, [C, C])
            c_nb1A = load_const(d_nb1A, [C, C])
            c_neg63 = load_const(d_neg63, [C, C])
            c_negall = load_const(d_negall, [C, C])
            c_negcol = load_const(d_negcol, [C, 1])
            c_onesrb = load_const(d_onesrb, [1, C], BF16)
            c_maskJ = load_const(d_maskJ, [C, C], BF16)
            c_identb = load_const(d_identb, [128, 128], BF16)
            c_identf = load_const(d_identf, [128, 128], F32)
            c_wf2 = load_const(d_wf2, [DV, HL * M], BF16)
            c_wg2 = load_const(d_wg2, [DV, HL * DV], BF16)
            c_bg2 = load_const(d_bg2, [1, HL * DV], BF16)
            c_cdiag = load_const(d_cdiag, [128, 32, 128], BF16)
            c_eps6 = pp.tile([C, 1], F32, tag="c_eps6")
            nc.vector.memset(c_eps6[:], 1e-6)

            # ---------- X^T + Wo ----------
            xt_sb = pp.tile([128, NKT, T], BF16, tag="xt_sb")
            xtr = d_xt.rearrange("(k p) t -> k p t", p=128)
            for kt in range(NKT):
                nc.sync.dma_start(xt_sb[:, kt, :], xtr[kt])

            wo_sb = pp.tile([128, HL, HID], BF16, tag="wo_sb")
            wor = d_wo.rearrange("(h p) o -> h p o", p=128)
            for h in range(HL):
                nc.sync.dma_start(wo_sb[:, h, :], wor[h])

            # ---------- P1: projections + conv (silu deferred) ----------
            qT = pp.tile([128, HL, T], BF16, tag="qT")
            kT = pp.tile([128, HL, T], BF16, tag="kT")
            vTc = pp.tile([128, HL, T], BF16, tag="vTc")
            wTc = pp.tile([128, HL, T], BF16, tag="wTc")

            def project_conv(d_w, pi, out_tile):
                w_sb = wsp.tile([128, NKT, HL * 128], BF16, tag="w_load")
                wr = d_w.rearrange("(k p) c -> k p c", p=128)
                for kt in range(NKT):
                    nc.sync.dma_start(w_sb[:, kt, :], wr[kt])
                for ct in range(HL):
                    xpad = xpp.tile([128, T + KW - 1], BF16, tag="xpad")
                    nc.vector.memset(xpad[:, 0:KW - 1], 0.0)
                    for tt in range(2):
                        ps = ps_big.tile([128, 512], F32, tag="pp")
                        for kt in range(NKT):
                            nc.tensor.matmul(
                                ps[:],
                                w_sb[:, kt, ct * 128:(ct + 1) * 128],
                                xt_sb[:, kt, tt * 512:(tt + 1) * 512],
                                start=(kt == 0), stop=(kt == NKT - 1),
                            )
                        dst = xpad[:, KW - 1 + tt * 512: KW - 1 + (tt + 1) * 512]
                        if tt == 0:
                            nc.scalar.copy(dst, ps[:])
                        else:
                            nc.vector.tensor_copy(dst, ps[:])
                    # conv as 4 accumulated diag matmuls per half, silu inline
                    for tt in range(2):
                        cps = ps_big.tile([128, 512], F32, tag="pp")
                        for i in range(KW):
                            nc.tensor.matmul(
                                cps[:],
                                c_cdiag[:, (pi * HL + ct) * KW + i, :],
                                xpad[:, tt * 512 + i: tt * 512 + i + 512],
                                start=(i == 0), stop=(i == KW - 1))
                        nc.scalar.activation(
                            out_tile[:, ct, tt * 512:(tt + 1) * 512],
                            cps[:], AF.Silu)

            project_conv(d_wq, 0, qT)
            project_conv(d_wk, 1, kT)
            project_conv(d_wv, 2, vTc)
            project_conv(d_ww, 3, wTc)

            # ---------- f1T, g1T (no conv) ----------
            def proj128T(d_w, tag):
                out = pp.tile([128, T], BF16, tag=tag)
                w_sb = wsp.tile([128, NKT, 128], BF16, tag="w_load")
                wr = d_w.rearrange("(k p) c -> k p c", p=128)
                for kt in range(NKT):
                    nc.sync.dma_start(w_sb[:, kt, :], wr[kt])
                for tt in range(2):
                    ps = ps_big.tile([128, 512], F32, tag="pp")
                    for kt in range(NKT):
                        nc.tensor.matmul(
                            ps[:], w_sb[:, kt, :],
                            xt_sb[:, kt, tt * 512:(tt + 1) * 512],
                            start=(kt == 0), stop=(kt == NKT - 1))
                    if tt == 0:
                        nc.scalar.copy(out[:, tt * 512:(tt + 1) * 512], ps[:])
                    else:
                        nc.vector.tensor_copy(out[:, tt * 512:(tt + 1) * 512], ps[:])
                return out

            f1T = proj128T(d_wf1, "f1T")
            g1T = proj128T(d_wg1, "g1T")

            # ---------- beta ----------
            betaT = pp.tile([HL, T], F32, tag="betaT")
            wb_sb = wsp.tile([128, NKT, HL], BF16, tag="wb_load")
            wbr = d_wb.rearrange("(k p) c -> k p c", p=128)
            for kt in range(NKT):
                nc.sync.dma_start(wb_sb[:, kt, :], wbr[kt])
            beta_ps = []
            for tt in range(2):
                ps = ps_big.tile([128, 512], F32, tag="pp")
                for kt in range(NKT):
                    nc.tensor.matmul(
                        ps[0:HL, :], wb_sb[:, kt, :],
                        xt_sb[:, kt, tt * 512:(tt + 1) * 512],
                        start=(kt == 0), stop=(kt == NKT - 1))
                beta_ps.append(ps)

            # ---------- gate (matmul now; sigmoid grouped below) ----------
            gate_sb = pp.tile([128, NCH, HL * DV], BF16, tag="gate_sb")
            gate_ps = []
            for n in range(NCH):
                ps = ps_cum.tile([128, HL * DV], F32, tag="cum")
                nc.tensor.matmul(ps[:], g1T[:, n * C:(n + 1) * C], c_wg2[:],
                                 start=True, stop=False)
                nc.tensor.matmul(ps[:], c_onesrb[:], c_bg2[:],
                                 start=False, stop=True)
                gate_ps.append(ps)
            # sigmoid phase: beta then gate (one table load)
            for tt in range(2):
                nc.scalar.activation(betaT[:, tt * 512:(tt + 1) * 512],
                                     beta_ps[tt][0:HL, :], AF.Sigmoid)
            for n in range(NCH):
                nc.scalar.activation(gate_sb[:, n, :], gate_ps[n][:], AF.Sigmoid)

            # ---------- gpos = ln(1 + exp(-s)), exp/ln phases grouped ----------
            gpos_sb = pp.tile([128, NCH, HL * M], F32, tag="gpos_sb")
            gpe_sb = pp.tile([128, NCH, HL * M], F32, tag="gpe_sb")
            for n in range(NCH):
                ps = ps_cum.tile([128, HL * M], F32, tag="cum")
                nc.tensor.matmul(ps[:], f1T[:, n * C:(n + 1) * C], c_wf2[:],
                                 start=True, stop=True)
                nc.scalar.activation(gpe_sb[:, n, :], ps[:], AF.Exp,
                                     scale=-1.0)
            nc.scalar.activation(gpos_sb[:], gpe_sb[:], AF.Ln, bias=1.0)

            # ---------- beta transpose -> [t, HL] ----------
            beta_t = pp.tile([128, NCH, HL], F32, tag="beta_t")
            for n in range(NCH):
                ps = ps_big.tile([128, HL], F32, tag="pp")
                nc.tensor.matmul(ps[:], betaT[:, n * C:(n + 1) * C],
                                 c_identf[0:HL, 0:HL], is_transpose=True)
                nc.scalar.copy(beta_t[:, n, :], ps[:])

            # ---------- W transpose + l2norm + beta -> bw ----------
            bw2 = pp.tile([128, NCH, HL * M], BF16, tag="bw2")
            ssq_all = pp.tile([128, NCH, HL], F32, tag="ssq_all")
            lnr_all = pp.tile([128, NCH, HL], F32, tag="lnr_all")
            rs_all = pp.tile([128, NCH, HL], F32, tag="rs_all")
            rsb_all = pp.tile([128, NCH, HL], F32, tag="rsb_all")
            Wc_all = pp.tile([128, NCH, HL * M], BF16, tag="Wc_all")
            for n in range(NCH):
                for h in range(HL):
                    psw = ps_tr.tile([128, 128], BF16, tag="trb")
                    nc.tensor.transpose(psw[:], wTc[:, h, n * C:(n + 1) * C],
                                        c_identb[:])
                    wc = Wc_all[:, n, h * M:(h + 1) * M]
                    nc.vector.tensor_copy(wc, psw[:])
                    junk = scC.tile([128, 128], BF16, tag="junkw")
                    nc.vector.scalar_tensor_tensor(
                        junk[:], wc, 1.0, wc,
                        op0=ALU.mult, op1=ALU.mult,
                        accum_out=ssq_all[:, n, h:h + 1])
            nc.scalar.activation(lnr_all[:], ssq_all[:], AF.Ln, bias=c_eps6[:])
            nc.scalar.activation(rs_all[:], lnr_all[:], AF.Exp, scale=-0.5)
            nc.vector.tensor_mul(rsb_all[:], rs_all[:], beta_t[:])
            for n in range(NCH):
                for h in range(HL):
                    nc.vector.tensor_scalar_mul(
                        bw2[:, n, h * M:(h + 1) * M],
                        Wc_all[:, n, h * M:(h + 1) * M],
                        rsb_all[:, n, h:h + 1])

            # ---------- states ----------
            Sk_cur = [None, None]
            Sv_cur = [None, None]
            Skb = [None, None]
            Svb = [None, None]

            den_sb = pp.tile([128, NCH, HL], F32, tag="den_sb")
            o_sb = pp.tile([128, NCH, HL, DV], BF16, tag="o_sb")
            oT_sb = pp.tile([128, HL, T], BF16, tag="oT_sb")

            # ---------- main chunk loop ----------
            for n in range(NCH):
                t0 = n * C
                tsl = slice(t0, t0 + C)
                # --- gate cumsums (both heads at once) ---
                cum_ps = ps_cum.tile([128, HL * M], F32, tag="cum")
                nc.tensor.matmul(cum_ps[:], c_trineg[:], gpos_sb[:, n, :],
                                 start=True, stop=True)
                Gc = scA.tile([128, HL * M], F32, tag="Gc")
                nc.scalar.copy(Gc[:], cum_ps[:])
                b1A_ps = ps_cum.tile([128, HL * M], F32, tag="cum")
                nc.tensor.matmul(b1A_ps[:], c_nb1A[:], gpos_sb[:, n, :],
                                 start=True, stop=True)
                b63_ps = ps_cum.tile([128, HL * M], F32, tag="cum")
                nc.tensor.matmul(b63_ps[:], c_neg63[:], gpos_sb[:, n, :],
                                 start=True, stop=True)
                blast_ps = ps_cum.tile([128, HL * M], F32, tag="cum")
                nc.tensor.matmul(blast_ps[:], c_negall[:], gpos_sb[:, n, :],
                                 start=True, stop=True)
                GcpA = scA.tile([128, HL * M], F32, tag="GcpA")
                nc.vector.tensor_sub(GcpA[:], Gc[:], b1A_ps[:])
                Gcp63 = scA.tile([128, HL * M], F32, tag="Gcp63")
                nc.vector.tensor_sub(Gcp63[:], Gc[:], b63_ps[:])
                grev = scA.tile([128, HL * M], F32, tag="grev")
                nc.vector.tensor_sub(grev[:], blast_ps[:], Gc[:])
                # LamCc per head: exp of per-slot total decay as a column
                lcc = [None, None]
                for h in range(HL):
                    hs = slice(h * M, (h + 1) * M)
                    lcc_ps = ps_cum.tile([128, 1], F32, tag="cum")
                    nc.tensor.matmul(lcc_ps[0:M, :], gpos_sb[:, n, hs],
                                     c_negcol[:], start=True, stop=True)
                    lcv = tnp.tile([M, 1], F32, tag="lcc_sb")
                    nc.scalar.activation(lcv[:], lcc_ps[0:M, :], AF.Exp)
                    lcc[h] = lcv
                # --- exps (all on exp table) ---
                Lam = scB.tile([128, HL * M], BF16, tag="Lam")
                nc.scalar.activation(Lam[:], Gc[:], AF.Exp)
                EposA = scB.tile([128, HL * M], BF16, tag="EposA")
                nc.scalar.activation(EposA[:], GcpA[:], AF.Exp)
                Epos63 = scB.tile([128, HL * M], BF16, tag="Epos63")
                nc.scalar.activation(Epos63[:], Gcp63[:], AF.Exp)
                EnegAe = scB.tile([128, HL * M], BF16, tag="EnegAe")
                nc.scalar.activation(EnegAe[:], GcpA[:], AF.Exp, scale=-1.0)
                Eneg63e = scB.tile([128, HL * M], BF16, tag="Eneg63e")
                nc.scalar.activation(Eneg63e[:], Gcp63[:], AF.Exp, scale=-1.0)
                Ereve = scB.tile([128, HL * M], BF16, tag="Ereve")
                nc.scalar.activation(Ereve[:], grev[:], AF.Exp)
                LamCb = scA.tile([128, HL * M], F32, tag="LamCb")
                nc.scalar.activation(LamCb[:], blast_ps[:], AF.Exp)
                # --- bw muls ---
                EnegA = scB.tile([128, HL * M], BF16, tag="EnegA")
                nc.vector.tensor_mul(EnegA[:], EnegAe[:], bw2[:, n, :])
                Eneg63 = scB.tile([128, HL * M], BF16, tag="Eneg63")
                nc.vector.tensor_mul(Eneg63[:], Eneg63e[:], bw2[:, n, :])
                Kdec = scB.tile([128, HL * M], BF16, tag="Kdec")
                nc.gpsimd.tensor_mul(Kdec[:], Ereve[:], bw2[:, n, :])

                for h in range(HL):
                    hs = slice(h * M, (h + 1) * M)
                    hv = slice(h * DV, (h + 1) * DV)
                    # --- K/V transposes for this chunk ---
                    psk = ps_tr.tile([128, 128], BF16, tag="trb")
                    nc.tensor.transpose(psk[:], kT[:, h, tsl], c_identb[:])
                    Kc = scC.tile([128, 128], BF16, tag="Kc")
                    nc.scalar.copy(Kc[:], psk[:])
                    psv = ps_tr.tile([128, 128], BF16, tag="trb")
                    nc.tensor.transpose(psv[:], vTc[:, h, tsl], c_identb[:])
                    Vc = scC.tile([128, 128], BF16, tag="Vc")
                    nc.scalar.copy(Vc[:], psv[:])
                    # --- Eneg transposes ---
                    pse = ps_tr.tile([128, 128], BF16, tag="trb")
                    nc.tensor.transpose(pse[:], EnegA[:, hs], c_identb[:])
                    EnegAT = scC.tile([128, 128], BF16, tag="EnegAT")
                    nc.vector.tensor_copy(EnegAT[:], pse[:])
                    ps6 = ps_tr.tile([128, 64], BF16, tag="trb")
                    nc.tensor.transpose(ps6[:], Eneg63[0:64, hs],
                                        c_identb[0:64, 0:64])
                    En63Tu = scC.tile([128, 64], BF16, tag="En63Tu")
                    nc.scalar.copy(En63Tu[:], ps6[:])
                    # --- pt + mask ---
                    pt_ps = ps_sc.tile([128, 128], F32, tag="sc")
                    nc.tensor.matmul(pt_ps[:], kT[:, h, tsl], qT[:, h, tsl],
                                     start=True, stop=True)
                    Ptm = scC.tile([128, 128], BF16, tag="Ptm")
                    nc.vector.scalar_tensor_tensor(
                        Ptm[:], pt_ps[:], SCALE, c_maskJ[:],
                        op0=ALU.mult, op1=ALU.mult)
                    # --- intra + s2 ---
                    intra_ps = ps_sc.tile([128, 128], F32, tag="sc")
                    nc.tensor.matmul(intra_ps[:], Ptm[:], Eneg63[:, hs],
                                     start=True, stop=True)
                    s2 = scD.tile([128, 128], F32, tag="s2")
                    nc.vector.tensor_mul(s2[:], intra_ps[:], Epos63[:, hs])
                    # --- scores ---
                    if n == 0:
                        sS = s2
                    else:
                        qs_ps = ps_sc.tile([128, 128], F32, tag="sc")
                        nc.tensor.matmul(qs_ps[:], qT[:, h, tsl], Skb[h][:],
                                         start=True, stop=True)
                        s1 = scD.tile([128, 128], F32, tag="s1")
                        nc.vector.scalar_tensor_tensor(
                            s1[:], qs_ps[:], SCALE, Lam[:, hs],
                            op0=ALU.mult, op1=ALU.mult)
                        sS = scD.tile([128, 128], F32, tag="sS")
                        nc.vector.tensor_add(sS[:], s1[:], s2[:])
                    nmx = tnp.tile([128, 1], F32, tag="nmx")
                    nc.vector.tensor_reduce(nmx[:], sS[:], AX.X, ALU.max,
                                            negate=True)
                    pexp = scC.tile([128, 128], BF16, tag="pexp")
                    nc.scalar.activation(pexp[:], sS[:], AF.Exp, bias=nmx[:],
                                         accum_out=den_sb[:, n, h:h + 1])
                    # --- attention weights ---
                    aL = scC.tile([128, 128], BF16, tag="aL")
                    nc.vector.tensor_mul(aL[:], pexp[:], Lam[:, hs])
                    aEA = scC.tile([128, 128], BF16, tag="aEA")
                    nc.vector.tensor_mul(aEA[:], pexp[:], EposA[:, hs])
                    aE63u = scC.tile([64, 128], BF16, tag="aE63u")
                    nc.gpsimd.tensor_mul(aE63u[:], pexp[64:128, :],
                                         Epos63[64:128, hs])
                    # transposes
                    psl = ps_tr.tile([128, 128], BF16, tag="trb")
                    nc.tensor.transpose(psl[:], aL[:], c_identb[:])
                    aLT = scC.tile([128, 128], BF16, tag="aLT")
                    nc.scalar.copy(aLT[:], psl[:])
                    psa = ps_tr.tile([128, 128], BF16, tag="trb")
                    nc.tensor.transpose(psa[:], aEA[:], c_identb[:])
                    aEAT = scC.tile([128, 128], BF16, tag="aEAT")
                    nc.vector.tensor_copy(aEAT[:], psa[:])
                    ps63 = ps_tr.tile([128, 64], BF16, tag="trb")
                    nc.tensor.transpose(ps63[:], aE63u[:],
                                        c_identb[0:64, 0:64])
                    aE63uT = scC.tile([128, 64], BF16, tag="aE63uT")
                    nc.scalar.copy(aE63uT[:], ps63[:])
                    # --- rt blocks ---
                    rt_ps = ps_sc.tile([128, 128], F32, tag="sc")
                    nc.tensor.matmul(rt_ps[0:64, 0:64], EnegAT[:, 0:64],
                                     aEAT[:, 0:64], start=True, stop=True)
                    nc.tensor.matmul(rt_ps[64:128, 64:128], EnegAT[:, 64:128],
                                     aEAT[:, 64:128], start=True, stop=True)
                    nc.tensor.matmul(rt_ps[0:64, 64:128], En63Tu[:],
                                     aE63uT[:], start=True, stop=True)
                    Rmt = scC.tile([128, 128], BF16, tag="Rmt")
                    nc.vector.memset(Rmt[64:128, 0:64], 0.0)
                    nc.vector.tensor_mul(Rmt[0:64, :], rt_ps[0:64, :],
                                         c_maskJ[0:64, :])
                    nc.vector.tensor_mul(Rmt[64:128, 64:128],
                                         rt_ps[64:128, 64:128],
                                         c_maskJ[64:128, 64:128])
                    # --- output ---
                    o_ps = ps_sc.tile([128, 128], F32, tag="sc")
                    if n == 0:
                        nc.tensor.matmul(o_ps[:], Rmt[:], Vc[:],
                                         start=True, stop=True)
                    else:
                        nc.tensor.matmul(o_ps[:], aLT[:], Svb[h][:],
                                         start=True, stop=False)
                        nc.tensor.matmul(o_ps[:], Rmt[:], Vc[:],
                                         start=False, stop=True)
                    nc.scalar.copy(o_sb[:, n, h, :], o_ps[:])
                    # --- state update (skip at last chunk) ---
                    if n < NCH - 1:
                        skk_ps = ps_sc.tile([128, 128], F32, tag="sc")
                        nc.tensor.matmul(skk_ps[:], Kc[:], Kdec[:, hs],
                                         start=True, stop=True)
                        svk_ps = ps_sc.tile([128, 128], F32, tag="sc")
                        nc.tensor.matmul(svk_ps[:], Kdec[:, hs], Vc[:],
                                         start=True, stop=True)
                        Sk_new = stp.tile([128, 128], F32, tag=f"Sk{h}")
                        Sv_new = stp.tile([128, 128], F32, tag=f"Sv{h}")
                        if n == 0:
                            nc.vector.tensor_copy(Sk_new[:], skk_ps[:])
                            nc.vector.tensor_copy(Sv_new[:], svk_ps[:])
                        else:
                            skt = scD.tile([128, 128], F32, tag="skt")
                            nc.vector.tensor_mul(skt[:], Sk_cur[h][:],
                                                 LamCb[:, hs])
                            nc.vector.tensor_add(Sk_new[:], skt[:], skk_ps[:])
                            nc.vector.scalar_tensor_tensor(
                                Sv_new[:], Sv_cur[h][:], lcc[h][:], svk_ps[:],
                                op0=ALU.mult, op1=ALU.add)
                        Sk_cur[h] = Sk_new
                        Sv_cur[h] = Sv_new
                        Skb_new = stp.tile([128, 128], BF16, tag=f"Skb{h}")
                        nc.gpsimd.tensor_copy(Skb_new[:], Sk_new[:])
                        Svb_new = stp.tile([128, 128], BF16, tag=f"Svb{h}")
                        nc.gpsimd.tensor_copy(Svb_new[:], Sv_new[:])
                        Skb[h] = Skb_new
                        Svb[h] = Svb_new

            # ---------- P6: epilogue (RMSNorm w/ den^2, gate, transpose) ----------
            osum_all = pp.tile([128, NCH, HL], F32, tag="osum_all")
            d2_all = pp.tile([128, NCH, HL], F32, tag="d2_all")
            ox_all = pp.tile([128, NCH, HL], F32, tag="ox_all")
            orm_all = pp.tile([128, NCH, HL], F32, tag="orm_all")
            orr_all = pp.tile([128, NCH, HL], F32, tag="orr_all")
            for n in range(NCH):
                for h in range(HL):
                    junk2 = scC.tile([128, 128], BF16, tag="junko")
                    nc.vector.scalar_tensor_tensor(
                        junk2[:], o_sb[:, n, h, :], 1.0, o_sb[:, n, h, :],
                        op0=ALU.mult, op1=ALU.mult,
                        accum_out=osum_all[:, n, h:h + 1])
            # d2 = EPS * den^2 ; ox = osum/DV + d2 ; orr = exp(-0.5*ln(ox))
            nc.vector.scalar_tensor_tensor(
                d2_all[:], den_sb[:], EPS, den_sb[:],
                op0=ALU.mult, op1=ALU.mult)
            nc.vector.scalar_tensor_tensor(
                ox_all[:], osum_all[:], 1.0 / DV, d2_all[:],
                op0=ALU.mult, op1=ALU.add)
            nc.scalar.activation(orm_all[:], ox_all[:], AF.Ln)
            nc.scalar.activation(orr_all[:], orm_all[:], AF.Exp, scale=-0.5)
            for n in range(NCH):
                for h in range(HL):
                    of = scC.tile([128, 128], BF16, tag="of")
                    nc.vector.scalar_tensor_tensor(
                        of[:], o_sb[:, n, h, :], orr_all[:, n, h:h + 1],
                        gate_sb[:, n, h * DV:(h + 1) * DV],
                        op0=ALU.mult, op1=ALU.mult)
                    pso = ps_tr.tile([128, 128], BF16, tag="trb")
                    nc.tensor.transpose(pso[:], of[:], c_identb[:])
                    nc.scalar.copy(oT_sb[:, h, n * C:(n + 1) * C], pso[:])

            # ---------- Wo partial + DMA out ----------
            for tt in range(NCH):
                for cl in range(4):
                    ps = ps_big.tile([128, 512], F32, tag="pp")
                    for h in range(HL):
                        nc.tensor.matmul(
                            ps[:], oT_sb[:, h, tt * C:(tt + 1) * C],
                            wo_sb[:, h, cl * 512:(cl + 1) * 512],
                            start=(h == 0), stop=(h == HL - 1))
                    osb = scC.tile([128, 512], BF16, tag="outsb")
                    if cl % 2 == 0:
                        nc.scalar.copy(osb[:], ps[:])
                    else:
                        nc.vector.tensor_copy(osb[:], ps[:])
                    nc.sync.dma_start(
                        d_out[tt * 128:(tt + 1) * 128, cl * 512:(cl + 1) * 512],
                        osb[:])
    nc.compile()
    return nc


def _host_inputs(inputs):
    f32 = np.float32
    bf16 = ml_dtypes.bfloat16
    X = np.ascontiguousarray(np.asarray(inputs["hidden_states"], f32)[0])
    XT = np.ascontiguousarray(X.T).astype(bf16)

    trineg = np.triu(np.full((C, C), -1.0, f32))
    nb1a = np.zeros((C, C), f32)     # [j, p]: -1 if (p<64, j<=31) or (p>=64, j<=95)
    nb1a[0:32, 0:64] = -1.0
    nb1a[0:96, 64:128] = -1.0
    neg63 = np.zeros((C, C), f32)
    neg63[0:64, :] = -1.0
    negall = np.full((C, C), -1.0, f32)
    negcol = np.full((C, 1), -1.0, f32)
    onesr = np.ones((1, C), f32)
    maskJ = np.triu(np.ones((C, C), f32))
    ident = np.eye(128, dtype=f32)

    Wo_full = np.asarray(inputs["Wo"], f32) * np.tile(
        np.asarray(inputs["norm_w"], f32), H)[:, None]

    def cdiag_for(c):
        # [128, 32, 128] bf16: diag(conv weight) per (proj, ct, tap)
        out = np.zeros((128, 32, 128), f32)
        idx = np.arange(128)
        for pi, key in enumerate(["cq", "ck", "cv", "cv"]):
            cw = np.asarray(inputs[key], f32)
            if pi == 3:
                pass  # w-conv uses cv weights (faithful to reference)
            for ct in range(HL):
                ch = (c * HL + ct) * 128
                for i in range(KW):
                    out[idx, (pi * HL + ct) * KW + i, idx] = cw[ch + idx, i]
        return out.astype(bf16)

    in_maps = []
    for c in range(8):
        hsl = slice(c * HL * 128, (c + 1) * HL * 128)
        bsl = slice(c * HL, (c + 1) * HL)
        m = {
            "xt": XT,
            "wq": np.asarray(inputs["Wq"], f32)[:, hsl].astype(bf16),
            "wk": np.asarray(inputs["Wk"], f32)[:, hsl].astype(bf16),
            "wv": np.asarray(inputs["Wv"], f32)[:, hsl].astype(bf16),
            "ww": np.asarray(inputs["Ww"], f32)[:, hsl].astype(bf16),
            "wf1": np.asarray(inputs["Wf1"], f32).astype(bf16),
            "wg1": np.asarray(inputs["Wg1"], f32).astype(bf16),
            "wb": np.asarray(inputs["Wb"], f32)[:, bsl].astype(bf16),
            "wf2": np.ascontiguousarray(
                np.asarray(inputs["Wf2"], f32)[:, hsl]).astype(bf16),
            "wg2": np.ascontiguousarray(
                np.asarray(inputs["Wg2"], f32)[:, hsl]).astype(bf16),
            "bg2": np.ascontiguousarray(
                np.asarray(inputs["bg2"], f32)[None, hsl]).astype(bf16),
            "wo": np.ascontiguousarray(Wo_full[hsl]).astype(bf16),
            "cdiag": cdiag_for(c),
            "trineg": trineg, "nb1a": nb1a, "neg63": neg63, "negall": negall,
            "negcol": negcol,
            "onesrb": onesr.astype(bf16), "maskj": maskJ.astype(bf16),
            "identb": ident.astype(bf16), "identf": ident,
        }
        in_maps.append(m)
    return in_maps


def kernel(_trace=False, **inputs):
    if "nc" not in _CACHE:
        _CACHE["nc"] = _build_nc()
    nc = _CACHE["nc"]
    in_maps = _host_inputs(inputs)
    res = run_bass_kernel_spmd(nc, in_maps, core_ids=list(range(8)),
                               trace=_trace)
    _CACHE["last_result"] = res
    out = np.zeros((T, HID), np.float32)
    for r in res.results:
        out += np.asarray(r["out"], np.float32)
    return out.reshape(B, T, HID)


# revision 17
# speedup vs baseline: 3.2359x; 1.0909x over previous
"""GatedSlotAttention2 Trainium2 Bass kernel (v2).

Sharding: 2 heads per core x 8 cores (H=16). Each core runs the full
pipeline for its two heads and emits a partial Wo product; the host sums
the 8 bf16 partials in f32.

v2 redesign vs baseline:
- C=128 chunks (8 serial steps instead of 16), three-block rt matmul with
  per-block gate offsets for overflow safety.
- bf16 matmuls everywhere off the state-accumulation path (4x PE rate).
- Denominator-free softmax: RMSNorm is scale-invariant; the dropped den
  is folded into the RMSNorm eps term (eps*den^2) via the Ln bias.
- Activation-table discipline: silu/sigmoid/softplus/ln/exp phases are
  grouped so the scalar engine loads each table O(1) times.
- Everything off the 8-step state recurrence is batched outside the
  serial chain; copies spread across scalar/gpsimd engines.
"""
import numpy as np
import ml_dtypes

import concourse.bass as bass
import concourse.bacc as bacc_mod
import concourse.mybir as mybir
import concourse.tile as tile
from concourse.bass_utils import run_bass_kernel_spmd

F32 = mybir.dt.float32
BF16 = mybir.dt.bfloat16
AF = mybir.ActivationFunctionType
ALU = mybir.AluOpType
MS = bass.MemorySpace
AX = mybir.AxisListType

B, T, HID = 1, 1024, 2048
H, DK, DV, M, KW = 16, 128, 128, 128, 4
SCALE = DK ** -0.5
EPS = 1e-5
C = 128           # chunk length
NCH = T // C      # 8 chunks
NKT = HID // 128  # 16 contraction tiles
HL = 2            # heads per core
MID = 63

_CACHE = {}


def _build_nc():
    nc = bacc_mod.Bacc("TRN2")

    # ---------------- DRAM I/O ----------------
    d_xt = nc.dram_tensor("xt", [HID, T], BF16, kind="ExternalInput")
    d_wq = nc.dram_tensor("wq", [HID, HL * DK], BF16, kind="ExternalInput")
    d_wk = nc.dram_tensor("wk", [HID, HL * DK], BF16, kind="ExternalInput")
    d_wv = nc.dram_tensor("wv", [HID, HL * DV], BF16, kind="ExternalInput")
    d_ww = nc.dram_tensor("ww", [HID, HL * M], BF16, kind="ExternalInput")
    d_wf1 = nc.dram_tensor("wf1", [HID, DV], BF16, kind="ExternalInput")
    d_wg1 = nc.dram_tensor("wg1", [HID, DV], BF16, kind="ExternalInput")
    d_wb = nc.dram_tensor("wb", [HID, HL], BF16, kind="ExternalInput")
    d_wf2 = nc.dram_tensor("wf2", [DV, HL * M], BF16, kind="ExternalInput")
    d_wg2 = nc.dram_tensor("wg2", [DV, HL * DV], BF16, kind="ExternalInput")
    d_bg2 = nc.dram_tensor("bg2", [1, HL * DV], BF16, kind="ExternalInput")
    d_wo = nc.dram_tensor("wo", [HL * DV, HID], BF16, kind="ExternalInput")
    # conv weights as diagonal matrices: [proj(4) x ct(2) x tap(4)] of 128x128
    d_cdiag = nc.dram_tensor("cdiag", [128, 32, 128], BF16, kind="ExternalInput")
    # constants
    F32R = mybir.dt.float32r
    d_trineg = nc.dram_tensor("trineg", [C, C], F32R, kind="ExternalInput")  # -1 if j<=i
    d_gcpa = nc.dram_tensor("gcpa", [C, C], F32R, kind="ExternalInput")      # trineg - nb1a
    d_gcp63 = nc.dram_tensor("gcp63", [C, C], F32R, kind="ExternalInput")    # trineg - neg63
    d_grev = nc.dram_tensor("grev", [C, C], F32R, kind="ExternalInput")      # negall - trineg
    d_negall = nc.dram_tensor("negall", [C, C], F32R, kind="ExternalInput")  # all -1
    d_negcol = nc.dram_tensor("negcol", [C, 4], F32R, kind="ExternalInput")  # cols of -1
    d_onesrb = nc.dram_tensor("onesrb", [1, C], BF16, kind="ExternalInput")
    d_maskJ = nc.dram_tensor("maskj", [C, C], BF16, kind="ExternalInput")    # 1 if j<=i
    d_identb = nc.dram_tensor("identb", [128, 128], BF16, kind="ExternalInput")
    d_identf = nc.dram_tensor("identf", [128, 128], F32, kind="ExternalInput")

    d_out = nc.dram_tensor("out", [T, HID], BF16, kind="ExternalOutput")

    with tile.TileContext(nc) as tc:
        with (
            tc.tile_pool(name="persist", bufs=1) as pp,
            tc.tile_pool(name="wstage", bufs=2) as wsp,
            tc.tile_pool(name="cacc", bufs=2) as ccp,
            tc.tile_pool(name="xpad", bufs=2) as xpp,
            tc.tile_pool(name="scrA", bufs=2) as scA,      # [128,256] f32 rotators
            tc.tile_pool(name="scrB", bufs=3) as scB,      # [128,256] bf16 exps
            tc.tile_pool(name="scrC", bufs=4) as scC,      # [128,128] bf16 per-head
            tc.tile_pool(name="scrD", bufs=3) as scD,      # f32 [128,128] per-head
            tc.tile_pool(name="states", bufs=2) as stp,
            tc.tile_pool(name="tiny", bufs=3) as tnp,
            tc.tile_pool(name="ps_big", bufs=2, space=MS.PSUM) as ps_big,
            tc.tile_pool(name="ps_cum", bufs=2, space=MS.PSUM) as ps_cum,
            tc.tile_pool(name="ps_sc", bufs=2, space=MS.PSUM) as ps_sc,
            tc.tile_pool(name="ps_tr", bufs=2, space=MS.PSUM) as ps_tr,
        ):
            # ---------- constants to SBUF ----------
            def load_const(dram, shape, dtype=F32):
                t = pp.tile(shape, dtype, tag=dram.name + "_sb")
                nc.sync.dma_start(t[:], dram[:])
                return t

            c_trineg = load_const(d_trineg, [C, C], F32R)
            c_gcpa = load_const(d_gcpa, [C, C], F32R)
            c_gcp63 = load_const(d_gcp63, [C, C], F32R)
            c_grev = load_const(d_grev, [C, C], F32R)
            c_negall = load_const(d_negall, [C, C], F32R)
            c_negcol = load_const(d_negcol, [C, 4], F32R)
            c_onesrb = load_const(d_onesrb, [1, C], BF16)
            c_maskJ = load_const(d_maskJ, [C, C], BF16)
            c_identb = load_const(d_identb, [128, 128], BF16)
            c_identf = load_const(d_identf, [128, 128], F32)
            c_wf2 = load_const(d_wf2, [DV, HL * M], BF16)
            c_wg2 = load_const(d_wg2, [DV, HL * DV], BF16)
            c_bg2 = load_const(d_bg2, [1, HL * DV], BF16)
            c_cdiag = load_const(d_cdiag, [128, 32, 128], BF16)
            c_eps6 = pp.tile([C, 1], F32, tag="c_eps6")
            nc.vector.memset(c_eps6[:], 1e-6)

            # ---------- X^T + Wo ----------
            xt_sb = pp.tile([128, NKT, T], BF16, tag="xt_sb")
            xtr = d_xt.rearrange("(k p) t -> k p t", p=128)
            for kt in range(NKT):
                nc.sync.dma_start(xt_sb[:, kt, :], xtr[kt])

            wo_sb = pp.tile([128, HL, HID], BF16, tag="wo_sb")
            wor = d_wo.rearrange("(h p) o -> h p o", p=128)
            for h in range(HL):
                nc.sync.dma_start(wo_sb[:, h, :], wor[h])

            # ---------- P1: projections + conv (silu deferred) ----------
            qT = pp.tile([128, HL, T], BF16, tag="qT")
            kT = pp.tile([128, HL, T], BF16, tag="kT")
            vTc = pp.tile([128, HL, T], BF16, tag="vTc")
            wTc = pp.tile([128, HL, T], BF16, tag="wTc")

            def project_conv(d_w, pi, out_tile):
                w_sb = wsp.tile([128, NKT, HL * 128], BF16, tag="w_load")
                wr = d_w.rearrange("(k p) c -> k p c", p=128)
                for kt in range(NKT):
                    nc.sync.dma_start(w_sb[:, kt, :], wr[kt])
                for ct in range(HL):
                    xpad = xpp.tile([128, T + KW - 1], BF16, tag="xpad")
                    nc.vector.memset(xpad[:, 0:KW - 1], 0.0)
                    for tt in range(2):
                        ps = ps_big.tile([128, 512], F32, tag="pp")
                        for kt in range(NKT):
                            nc.tensor.matmul(
                                ps[:],
                                w_sb[:, kt, ct * 128:(ct + 1) * 128],
                                xt_sb[:, kt, tt * 512:(tt + 1) * 512],
                                start=(kt == 0), stop=(kt == NKT - 1),
                            )
                        dst = xpad[:, KW - 1 + tt * 512: KW - 1 + (tt + 1) * 512]
                        if tt == 0:
                            nc.scalar.copy(dst, ps[:])
                        else:
                            nc.vector.tensor_copy(dst, ps[:])
                    # conv as 4 accumulated diag matmuls per half, silu inline
                    for tt in range(2):
                        cps = ps_big.tile([128, 512], F32, tag="pp")
                        for i in range(KW):
                            nc.tensor.matmul(
                                cps[:],
                                c_cdiag[:, (pi * HL + ct) * KW + i, :],
                                xpad[:, tt * 512 + i: tt * 512 + i + 512],
                                start=(i == 0), stop=(i == KW - 1))
                        nc.scalar.activation(
                            out_tile[:, ct, tt * 512:(tt + 1) * 512],
                            cps[:], AF.Silu)

            project_conv(d_wq, 0, qT)
            project_conv(d_wk, 1, kT)
            project_conv(d_wv, 2, vTc)
            project_conv(d_ww, 3, wTc)

            # ---------- f1T, g1T (no conv) ----------
            def proj128T(d_w, tag):
                out = pp.tile([128, T], BF16, tag=tag)
                w_sb = wsp.tile([128, NKT, 128], BF16, tag="w_load")
                wr = d_w.rearrange("(k p) c -> k p c", p=128)
                for kt in range(NKT):
                    nc.sync.dma_start(w_sb[:, kt, :], wr[kt])
                for tt in range(2):
                    ps = ps_big.tile([128, 512], F32, tag="pp")
                    for kt in range(NKT):
                        nc.tensor.matmul(
                            ps[:], w_sb[:, kt, :],
                            xt_sb[:, kt, tt * 512:(tt + 1) * 512],
                            start=(kt == 0), stop=(kt == NKT - 1))
                    if tt == 0:
                        nc.scalar.copy(out[:, tt * 512:(tt + 1) * 512], ps[:])
                    else:
                        nc.vector.tensor_copy(out[:, tt * 512:(tt + 1) * 512], ps[:])
                return out

            f1T = proj128T(d_wf1, "f1T")
            g1T = proj128T(d_wg1, "g1T")

            # ---------- beta ----------
            betaT = pp.tile([HL, T], F32, tag="betaT")
            wb_sb = wsp.tile([128, NKT, HL], BF16, tag="wb_load")
            wbr = d_wb.rearrange("(k p) c -> k p c", p=128)
            for kt in range(NKT):
                nc.sync.dma_start(wb_sb[:, kt, :], wbr[kt])
            beta_ps = []
            for tt in range(2):
                ps = ps_big.tile([128, 512], F32, tag="pp")
                for kt in range(NKT):
                    nc.tensor.matmul(
                        ps[0:HL, :], wb_sb[:, kt, :],
                        xt_sb[:, kt, tt * 512:(tt + 1) * 512],
                        start=(kt == 0), stop=(kt == NKT - 1))
                beta_ps.append(ps)

            # ---------- gate (matmul now; sigmoid grouped below) ----------
            gate_sb = pp.tile([128, NCH, HL * DV], BF16, tag="gate_sb")
            gate_ps = []
            for n in range(NCH):
                ps = ps_cum.tile([128, HL * DV], F32, tag="cum")
                nc.tensor.matmul(ps[:], g1T[:, n * C:(n + 1) * C], c_wg2[:],
                                 start=True, stop=False)
                nc.tensor.matmul(ps[:], c_onesrb[:], c_bg2[:],
                                 start=False, stop=True)
                gate_ps.append(ps)
            # sigmoid phase: beta then gate (one table load)
            for tt in range(2):
                nc.scalar.activation(betaT[:, tt * 512:(tt + 1) * 512],
                                     beta_ps[tt][0:HL, :], AF.Sigmoid)
            for n in range(NCH):
                nc.scalar.activation(gate_sb[:, n, :], gate_ps[n][:], AF.Sigmoid)

            # ---------- gpos = ln(1 + exp(-s)), exp/ln phases grouped ----------
            gpos_sb = pp.tile([128, NCH, HL * M], F32R, tag="gpos_sb")
            gpe_sb = pp.tile([128, NCH, HL * M], F32, tag="gpe_sb")
            for n in range(NCH):
                ps = ps_cum.tile([128, HL * M], F32, tag="cum")
                nc.tensor.matmul(ps[:], f1T[:, n * C:(n + 1) * C], c_wf2[:],
                                 start=True, stop=True)
                nc.scalar.activation(gpe_sb[:, n, :], ps[:], AF.Exp,
                                     scale=-1.0)
            nc.scalar.activation(gpos_sb[:], gpe_sb[:], AF.Ln, bias=1.0)

            # ---------- beta transpose -> [t, HL] ----------
            beta_t = pp.tile([128, NCH, HL], F32, tag="beta_t")
            for n in range(NCH):
                ps = ps_big.tile([128, HL], F32, tag="pp")
                nc.tensor.matmul(ps[:], betaT[:, n * C:(n + 1) * C],
                                 c_identf[0:HL, 0:HL], is_transpose=True)
                nc.scalar.copy(beta_t[:, n, :], ps[:])

            # ---------- W transpose + l2norm + beta -> bw ----------
            bw2 = pp.tile([128, NCH, HL * M], BF16, tag="bw2")
            ssq_all = pp.tile([128, NCH, HL], F32, tag="ssq_all")
            lnr_all = pp.tile([128, NCH, HL], F32, tag="lnr_all")
            rs_all = pp.tile([128, NCH, HL], F32, tag="rs_all")
            rsb_all = pp.tile([128, NCH, HL], F32, tag="rsb_all")
            Wc_all = pp.tile([128, NCH, HL * M], BF16, tag="Wc_all")
            for n in range(NCH):
                for h in range(HL):
                    psw = ps_tr.tile([128, 128], BF16, tag="trb")
                    nc.tensor.transpose(psw[:], wTc[:, h, n * C:(n + 1) * C],
                                        c_identb[:])
                    wc = Wc_all[:, n, h * M:(h + 1) * M]
                    nc.vector.tensor_copy(wc, psw[:])
                    junk = scC.tile([128, 128], BF16, tag="junkw")
                    nc.vector.scalar_tensor_tensor(
                        junk[:], wc, 1.0, wc,
                        op0=ALU.mult, op1=ALU.mult,
                        accum_out=ssq_all[:, n, h:h + 1])
            nc.scalar.activation(lnr_all[:], ssq_all[:], AF.Ln, bias=c_eps6[:])
            nc.scalar.activation(rs_all[:], lnr_all[:], AF.Exp, scale=-0.5)
            nc.vector.tensor_mul(rsb_all[:], rs_all[:], beta_t[:])
            for n in range(NCH):
                for h in range(HL):
                    nc.vector.tensor_scalar_mul(
                        bw2[:, n, h * M:(h + 1) * M],
                        Wc_all[:, n, h * M:(h + 1) * M],
                        rsb_all[:, n, h:h + 1])

            # ---------- states ----------
            Sk_cur = [None, None]
            Sv_cur = [None, None]
            Skb = [None, None]
            Svb = [None, None]

            den_sb = pp.tile([128, NCH, HL], F32, tag="den_sb")
            o_sb = pp.tile([128, NCH, HL, DV], BF16, tag="o_sb")
            oT_sb = pp.tile([128, HL, T], BF16, tag="oT_sb")

            # ---------- main chunk loop ----------
            for n in range(NCH):
                t0 = n * C
                tsl = slice(t0, t0 + C)
                # --- gate cumsums (both heads at once) ---
                gsl = gpos_sb[:, n, :]
                gc_ps = ps_cum.tile([128, HL * M], F32, tag="cum")
                nc.tensor.matmul(gc_ps[:], c_trineg[:], gsl,
                                 start=True, stop=True)
                ga_ps = ps_cum.tile([128, HL * M], F32, tag="cum")
                nc.tensor.matmul(ga_ps[:], c_gcpa[:], gsl,
                                 start=True, stop=True)
                g6_ps = ps_cum.tile([128, HL * M], F32, tag="cum")
                nc.tensor.matmul(g6_ps[:], c_gcp63[:], gsl,
                                 start=True, stop=True)
                gr_ps = ps_cum.tile([128, HL * M], F32, tag="cum")
                nc.tensor.matmul(gr_ps[:], c_grev[:], gsl,
                                 start=True, stop=True)
                gl_ps = ps_cum.tile([128, HL * M], F32, tag="cum")
                nc.tensor.matmul(gl_ps[:], c_negall[:], gsl,
                                 start=True, stop=True)
                # LamCc per head: exp of per-slot total decay as a column
                lcc = [None, None]
                for h in range(HL):
                    hs = slice(h * M, (h + 1) * M)
                    lcc_ps = ps_cum.tile([128, 4], F32, tag="cum")
                    nc.tensor.matmul(lcc_ps[0:M, :], gsl[:, hs],
                                     c_negcol[:], start=True, stop=True)
                    lcv = tnp.tile([M, 1], F32, tag="lcc_sb")
                    nc.scalar.activation(lcv[:], lcc_ps[0:M, 0:1], AF.Exp)
                    lcc[h] = lcv
                # --- exps straight from PSUM (all on exp table) ---
                Lam = scB.tile([128, HL * M], BF16, tag="Lam")
                nc.scalar.activation(Lam[:], gc_ps[:], AF.Exp)
                EposA = scB.tile([128, HL * M], BF16, tag="EposA")
                nc.scalar.activation(EposA[:], ga_ps[:], AF.Exp)
                EnegAe = scB.tile([128, HL * M], BF16, tag="EnegAe")
                nc.scalar.activation(EnegAe[:], ga_ps[:], AF.Exp, scale=-1.0)
                Epos63 = scB.tile([128, HL * M], BF16, tag="Epos63")
                nc.scalar.activation(Epos63[:], g6_ps[:], AF.Exp)
                Eneg63e = scB.tile([128, HL * M], BF16, tag="Eneg63e")
                nc.scalar.activation(Eneg63e[:], g6_ps[:], AF.Exp, scale=-1.0)
                Ereve = scB.tile([128, HL * M], BF16, tag="Ereve")
                nc.scalar.activation(Ereve[:], gr_ps[:], AF.Exp)
                LamCb = scA.tile([128, HL * M], F32, tag="LamCb")
                nc.scalar.activation(LamCb[:], gl_ps[:], AF.Exp)
                # --- bw muls ---
                EnegA = scB.tile([128, HL * M], BF16, tag="EnegA")
                nc.vector.tensor_mul(EnegA[:], EnegAe[:], bw2[:, n, :])
                Eneg63 = scB.tile([128, HL * M], BF16, tag="Eneg63")
                nc.vector.tensor_mul(Eneg63[:], Eneg63e[:], bw2[:, n, :])
                Kdec = scB.tile([128, HL * M], BF16, tag="Kdec")
                nc.gpsimd.tensor_mul(Kdec[:], Ereve[:], bw2[:, n, :])

                for h in range(HL):
                    hs = slice(h * M, (h + 1) * M)
                    hv = slice(h * DV, (h + 1) * DV)
                    # --- K/V transposes for this chunk ---
                    psk = ps_tr.tile([128, 128], BF16, tag="trb")
                    nc.tensor.transpose(psk[:], kT[:, h, tsl], c_identb[:])
                    Kc = scC.tile([128, 128], BF16, tag="Kc")
                    nc.scalar.copy(Kc[:], psk[:])
                    psv = ps_tr.tile([128, 128], BF16, tag="trb")
                    nc.tensor.transpose(psv[:], vTc[:, h, tsl], c_identb[:])
                    Vc = scC.tile([128, 128], BF16, tag="Vc")
                    nc.scalar.copy(Vc[:], psv[:])
                    # --- Eneg transposes ---
                    pse = ps_tr.tile([128, 128], BF16, tag="trb")
                    nc.tensor.transpose(pse[:], EnegA[:, hs], c_identb[:])
                    EnegAT = scC.tile([128, 128], BF16, tag="EnegAT")
                    nc.vector.tensor_copy(EnegAT[:], pse[:])
                    ps6 = ps_tr.tile([128, 64], BF16, tag="trb")
                    nc.tensor.transpose(ps6[:], Eneg63[0:64, hs],
                                        c_identb[0:64, 0:64])
                    En63Tu = scC.tile([128, 64], BF16, tag="En63Tu")
                    nc.scalar.copy(En63Tu[:], ps6[:])
                    # --- pt + mask ---
                    pt_ps = ps_sc.tile([128, 128], F32, tag="sc")
                    nc.tensor.matmul(pt_ps[:], kT[:, h, tsl], qT[:, h, tsl],
                                     start=True, stop=True)
                    Ptm = scC.tile([128, 128], BF16, tag="Ptm")
                    nc.vector.scalar_tensor_tensor(
                        Ptm[:], pt_ps[:], SCALE, c_maskJ[:],
                        op0=ALU.mult, op1=ALU.mult)
                    # --- intra + s2 ---
                    intra_ps = ps_sc.tile([128, 128], F32, tag="sc")
                    nc.tensor.matmul(intra_ps[:], Ptm[:], Eneg63[:, hs],
                                     start=True, stop=True)
                    s2 = scD.tile([128, 128], F32, tag="s2")
                    nc.vector.tensor_mul(s2[:], intra_ps[:], Epos63[:, hs])
                    # --- scores ---
                    if n == 0:
                        sS = s2
                    else:
                        qs_ps = ps_sc.tile([128, 128], F32, tag="sc")
                        nc.tensor.matmul(qs_ps[:], qT[:, h, tsl], Skb[h][:],
                                         start=True, stop=True)
                        s1 = scD.tile([128, 128], F32, tag="s1")
                        nc.vector.scalar_tensor_tensor(
                            s1[:], qs_ps[:], SCALE, Lam[:, hs],
                            op0=ALU.mult, op1=ALU.mult)
                        sS = scD.tile([128, 128], F32, tag="sS")
                        nc.vector.tensor_add(sS[:], s1[:], s2[:])
                    nmx = tnp.tile([128, 1], F32, tag="nmx")
                    nc.vector.tensor_reduce(nmx[:], sS[:], AX.X, ALU.max,
                                            negate=True)
                    pexp = scC.tile([128, 128], BF16, tag="pexp")
                    nc.scalar.activation(pexp[:], sS[:], AF.Exp, bias=nmx[:],
                                         accum_out=den_sb[:, n, h:h + 1])
                    # --- attention weights ---
                    aL = scC.tile([128, 128], BF16, tag="aL")
                    nc.vector.tensor_mul(aL[:], pexp[:], Lam[:, hs])
                    aEA = scC.tile([128, 128], BF16, tag="aEA")
                    nc.vector.tensor_mul(aEA[:], pexp[:], EposA[:, hs])
                    aE63u = scC.tile([64, 128], BF16, tag="aE63u")
                    nc.gpsimd.tensor_mul(aE63u[:], pexp[64:128, :],
                                         Epos63[64:128, hs])
                    # transposes
                    psl = ps_tr.tile([128, 128], BF16, tag="trb")
                    nc.tensor.transpose(psl[:], aL[:], c_identb[:])
                    aLT = scC.tile([128, 128], BF16, tag="aLT")
                    nc.scalar.copy(aLT[:], psl[:])
                    psa = ps_tr.tile([128, 128], BF16, tag="trb")
                    nc.tensor.transpose(psa[:], aEA[:], c_identb[:])
                    aEAT = scC.tile([128, 128], BF16, tag="aEAT")
                    nc.vector.tensor_copy(aEAT[:], psa[:])
                    ps63 = ps_tr.tile([128, 64], BF16, tag="trb")
                    nc.tensor.transpose(ps63[:], aE63u[:],
                                        c_identb[0:64, 0:64])
                    aE63uT = scC.tile([128, 64], BF16, tag="aE63uT")
                    nc.scalar.copy(aE63uT[:], ps63[:])
                    # --- rt blocks ---
                    rt_ps = ps_sc.tile([128, 128], F32, tag="sc")
                    nc.tensor.matmul(rt_ps[0:64, 0:64], EnegAT[:, 0:64],
                                     aEAT[:, 0:64], start=True, stop=True)
                    nc.tensor.matmul(rt_ps[64:128, 64:128], EnegAT[:, 64:128],
                                     aEAT[:, 64:128], start=True, stop=True)
                    nc.tensor.matmul(rt_ps[0:64, 64:128], En63Tu[:],
                                     aE63uT[:], start=True, stop=True)
                    Rmt = scC.tile([128, 128], BF16, tag="Rmt")
                    nc.vector.memset(Rmt[64:128, 0:64], 0.0)
                    nc.vector.tensor_mul(Rmt[0:64, :], rt_ps[0:64, :],
                                         c_maskJ[0:64, :])
                    nc.vector.tensor_mul(Rmt[64:128, 64:128],
                                         rt_ps[64:128, 64:128],
                                         c_maskJ[64:128, 64:128])
                    # --- output ---
                    o_ps = ps_sc.tile([128, 128], F32, tag="sc")
                    if n == 0:
                        nc.tensor.matmul(o_ps[:], Rmt[:], Vc[:],
                                         start=True, stop=True)
                    else:
                        nc.tensor.matmul(o_ps[:], aLT[:], Svb[h][:],
                                         start=True, stop=False)
                        nc.tensor.matmul(o_ps[:], Rmt[:], Vc[:],
                                         start=False, stop=True)
                    nc.scalar.copy(o_sb[:, n, h, :], o_ps[:])
                    # --- state update (skip at last chunk) ---
                    if n < NCH - 1:
                        skk_ps = ps_sc.tile([128, 128], F32, tag="sc")
                        nc.tensor.matmul(skk_ps[:], Kc[:], Kdec[:, hs],
                                         start=True, stop=True)
                        svk_ps = ps_sc.tile([128, 128], F32, tag="sc")
                        nc.tensor.matmul(svk_ps[:], Kdec[:, hs], Vc[:],
                                         start=True, stop=True)
                        Sk_new = stp.tile([128, 128], F32, tag=f"Sk{h}")
                        Sv_new = stp.tile([128, 128], F32, tag=f"Sv{h}")
                        if n == 0:
                            nc.vector.tensor_copy(Sk_new[:], skk_ps[:])
                            nc.vector.tensor_copy(Sv_new[:], svk_ps[:])
                        else:
                            skt = scD.tile([128, 128], F32, tag="skt")
                            nc.vector.tensor_mul(skt[:], Sk_cur[h][:],
                                                 LamCb[:, hs])
                            nc.vector.tensor_add(Sk_new[:], skt[:], skk_ps[:])
                            nc.vector.scalar_tensor_tensor(
                                Sv_new[:], Sv_cur[h][:], lcc[h][:], svk_ps[:],
                                op0=ALU.mult, op1=ALU.add)
                        Sk_cur[h] = Sk_new
                        Sv_cur[h] = Sv_new
                        Skb_new = stp.tile([128, 128], BF16, tag=f"Skb{h}")
                        nc.gpsimd.tensor_copy(Skb_new[:], Sk_new[:])
                        Svb_new = stp.tile([128, 128], BF16, tag=f"Svb{h}")
                        nc.gpsimd.tensor_copy(Svb_new[:], Sv_new[:])
                        Skb[h] = Skb_new
                        Svb[h] = Svb_new

            # ---------- P6: epilogue (RMSNorm w/ den^2, gate, transpose) ----------
            osum_all = pp.tile([128, NCH, HL], F32, tag="osum_all")
            d2_all = pp.tile([128, NCH, HL], F32, tag="d2_all")
            ox_all = pp.tile([128, NCH, HL], F32, tag="ox_all")
            orm_all = pp.tile([128, NCH, HL], F32, tag="orm_all")
            orr_all = pp.tile([128, NCH, HL], F32, tag="orr_all")
            for n in range(NCH):
                for h in range(HL):
                    junk2 = scC.tile([128, 128], BF16, tag="junko")
                    nc.vector.scalar_tensor_tensor(
                        junk2[:], o_sb[:, n, h, :], 1.0, o_sb[:, n, h, :],
                        op0=ALU.mult, op1=ALU.mult,
                        accum_out=osum_all[:, n, h:h + 1])
            # d2 = EPS * den^2 ; ox = osum/DV + d2 ; orr = exp(-0.5*ln(ox))
            nc.vector.scalar_tensor_tensor(
                d2_all[:], den_sb[:], EPS, den_sb[:],
                op0=ALU.mult, op1=ALU.mult)
            nc.vector.scalar_tensor_tensor(
                ox_all[:], osum_all[:], 1.0 / DV, d2_all[:],
                op0=ALU.mult, op1=ALU.add)
            nc.scalar.activation(orm_all[:], ox_all[:], AF.Ln)
            nc.scalar.activation(orr_all[:], orm_all[:], AF.Exp, scale=-0.5)
            for n in range(NCH):
                for h in range(HL):
                    of = scC.tile([128, 128], BF16, tag="of")
                    nc.vector.scalar_tensor_tensor(
                        of[:], o_sb[:, n, h, :], orr_all[:, n, h:h + 1],
                        gate_sb[:, n, h * DV:(h + 1) * DV],
                        op0=ALU.mult, op1=ALU.mult)
                    pso = ps_tr.tile([128, 128], BF16, tag="trb")
                    nc.tensor.transpose(pso[:], of[:], c_identb[:])
                    nc.scalar.copy(oT_sb[:, h, n * C:(n + 1) * C], pso[:])

            # ---------- Wo partial + DMA out ----------
            for tt in range(NCH):
                for cl in range(4):
                    ps = ps_big.tile([128, 512], F32, tag="pp")
                    for h in range(HL):
                        nc.tensor.matmul(
                            ps[:], oT_sb[:, h, tt * C:(tt + 1) * C],
                            wo_sb[:, h, cl * 512:(cl + 1) * 512],
                            start=(h == 0), stop=(h == HL - 1))
                    osb = scC.tile([128, 512], BF16, tag="outsb")
                    if cl % 2 == 0:
                        nc.scalar.copy(osb[:], ps[:])
                    else:
                        nc.vector.tensor_copy(osb[:], ps[:])
                    nc.sync.dma_start(
                        d_out[tt * 128:(tt + 1) * 128, cl * 512:(cl + 1) * 512],
                        osb[:])
    nc.compile()
    return nc


def _host_inputs(inputs):
    f32 = np.float32
    bf16 = ml_dtypes.bfloat16
    X = np.ascontiguousarray(np.asarray(inputs["hidden_states"], f32)[0])
    XT = np.ascontiguousarray(X.T).astype(bf16)

    trineg = np.triu(np.full((C, C), -1.0, f32))
    nb1a = np.zeros((C, C), f32)     # [j, p]: -1 if (p<64, j<=31) or (p>=64, j<=95)
    nb1a[0:32, 0:64] = -1.0
    nb1a[0:96, 64:128] = -1.0
    neg63 = np.zeros((C, C), f32)
    neg63[0:64, :] = -1.0
    negall = np.full((C, C), -1.0, f32)
    negcol = np.full((C, 4), -1.0, f32)
    onesr = np.ones((1, C), f32)
    maskJ = np.triu(np.ones((C, C), f32))
    ident = np.eye(128, dtype=f32)
    gcpa = trineg - nb1a
    gcp63 = trineg - neg63
    grev_m = negall - trineg

    Wo_full = np.asarray(inputs["Wo"], f32) * np.tile(
        np.asarray(inputs["norm_w"], f32), H)[:, None]

    def cdiag_for(c):
        # [128, 32, 128] bf16: diag(conv weight) per (proj, ct, tap)
        out = np.zeros((128, 32, 128), f32)
        idx = np.arange(128)
        for pi, key in enumerate(["cq", "ck", "cv", "cv"]):
            cw = np.asarray(inputs[key], f32)
            if pi == 3:
                pass  # w-conv uses cv weights (faithful to reference)
            for ct in range(HL):
                ch = (c * HL + ct) * 128
                for i in range(KW):
                    out[idx, (pi * HL + ct) * KW + i, idx] = cw[ch + idx, i]
        return out.astype(bf16)

    in_maps = []
    for c in range(8):
        hsl = slice(c * HL * 128, (c + 1) * HL * 128)
        bsl = slice(c * HL, (c + 1) * HL)
        m = {
            "xt": XT,
            "wq": np.asarray(inputs["Wq"], f32)[:, hsl].astype(bf16),
            "wk": np.asarray(inputs["Wk"], f32)[:, hsl].astype(bf16),
            "wv": np.asarray(inputs["Wv"], f32)[:, hsl].astype(bf16),
            "ww": np.asarray(inputs["Ww"], f32)[:, hsl].astype(bf16),
            "wf1": np.asarray(inputs["Wf1"], f32).astype(bf16),
            "wg1": np.asarray(inputs["Wg1"], f32).astype(bf16),
            "wb": np.asarray(inputs["Wb"], f32)[:, bsl].astype(bf16),
            "wf2": np.ascontiguousarray(
                np.asarray(inputs["Wf2"], f32)[:, hsl]).astype(bf16),
            "wg2": np.ascontiguousarray(
                np.asarray(inputs["Wg2"], f32)[:, hsl]).astype(bf16),
            "bg2": np.ascontiguousarray(
                np.asarray(inputs["bg2"], f32)[None, hsl]).astype(bf16),
            "wo": np.ascontiguousarray(Wo_full[hsl]).astype(bf16),
            "cdiag": cdiag_for(c),
            "trineg": trineg, "gcpa": gcpa, "gcp63": gcp63, "grev": grev_m,
            "negall": negall, "negcol": negcol,
            "onesrb": onesr.astype(bf16), "maskj": maskJ.astype(bf16),
            "identb": ident.astype(bf16), "identf": ident,
        }
        in_maps.append(m)
    return in_maps


def kernel(_trace=False, **inputs):
    if "nc" not in _CACHE:
        _CACHE["nc"] = _build_nc()
    nc = _CACHE["nc"]
    in_maps = _host_inputs(inputs)
    res = run_bass_kernel_spmd(nc, in_maps, core_ids=list(range(8)),
                               trace=_trace)
    _CACHE["last_result"] = res
    out = np.zeros((T, HID), np.float32)
    for r in res.results:
        out += np.asarray(r["out"], np.float32)
    return out.reshape(B, T, HID)


# revision 27
# speedup vs baseline: 3.8772x; 1.1982x over previous
"""GatedSlotAttention2 Trainium2 Bass kernel (v3).

Sharding: 2 heads per core x 8 cores (H=16). Each core runs the full
pipeline for its two heads and emits a partial Wo product; the host sums
the 8 bf16 partials in f32.

Design notes:
- C=128 chunks; three-block rt matmul with per-block gate offsets for
  overflow safety (diag blocks use G_31/G_95, lower-left uses G_63).
- Denominator-free softmax: RMSNorm is scale-invariant; the dropped den
  folds into the RMSNorm eps term (eps*den^2).
- bf16 matmuls off the gate-cumsum path; cumsums in f32r (fast path).
- Per-chunk offsets folded into the cumsum stationaries so exps read
  PSUM directly (no Gc copy, no DVE subs).
- States in bf16: per-chunk decay exp(sum g) ~ e^-90 underflows anyway,
  so an f32 master adds nothing.
- Emission phases: P1 projections (W,f1,beta,l2norm,gpos first so the
  scan's inputs finish earliest) -> phase B (all state-independent chunk
  work + the state chain, overlapping P1's PE shadow) -> phase C (16
  independent softmax waves) -> split epilogue halves + Wo (tail overlap).
- Activation-table discipline: silu/sigmoid/ln/exp grouped; l2norm and
  epilogue use single wide Ln/Exp instructions.
"""
import numpy as np
import ml_dtypes

import concourse.bass as bass
import concourse.bacc as bacc_mod
import concourse.mybir as mybir
import concourse.tile as tile
from concourse.bass_utils import run_bass_kernel_spmd

F32 = mybir.dt.float32
F32R = mybir.dt.float32r
BF16 = mybir.dt.bfloat16
AF = mybir.ActivationFunctionType
ALU = mybir.AluOpType
MS = bass.MemorySpace
AX = mybir.AxisListType

B, T, HID = 1, 1024, 2048
H, DK, DV, M, KW = 16, 128, 128, 128, 4
SCALE = DK ** -0.5
EPS = 1e-5
C = 128
NCH = T // C
NKT = HID // 128
HL = 2
MID = 63

_CACHE = {}


def _build_nc():
    nc = bacc_mod.Bacc("TRN2")

    # ---------------- DRAM I/O ----------------
    d_xt = nc.dram_tensor("xt", [HID, T], BF16, kind="ExternalInput")
    d_wq = nc.dram_tensor("wq", [HID, HL * DK], BF16, kind="ExternalInput")
    d_wk = nc.dram_tensor("wk", [HID, HL * DK], BF16, kind="ExternalInput")
    d_wv = nc.dram_tensor("wv", [HID, HL * DV], BF16, kind="ExternalInput")
    d_ww = nc.dram_tensor("ww", [HID, HL * M], BF16, kind="ExternalInput")
    d_wf1 = nc.dram_tensor("wf1", [HID, DV], BF16, kind="ExternalInput")
    d_wg1 = nc.dram_tensor("wg1", [HID, DV], BF16, kind="ExternalInput")
    d_wb = nc.dram_tensor("wb", [HID, HL], BF16, kind="ExternalInput")
    d_wf2 = nc.dram_tensor("wf2", [DV, HL * M], BF16, kind="ExternalInput")
    d_wg2 = nc.dram_tensor("wg2", [DV, HL * DV], BF16, kind="ExternalInput")
    d_bg2 = nc.dram_tensor("bg2", [1, HL * DV], BF16, kind="ExternalInput")
    d_wo = nc.dram_tensor("wo", [HL * DV, HID], BF16, kind="ExternalInput")
    d_cdiag = nc.dram_tensor("cdiag", [128, 32, 128], BF16, kind="ExternalInput")
    # constants
    d_trineg = nc.dram_tensor("trineg", [C, C], F32R, kind="ExternalInput")
    d_gcpa = nc.dram_tensor("gcpa", [C, C], F32R, kind="ExternalInput")
    d_gcp63 = nc.dram_tensor("gcp63", [C, C], F32R, kind="ExternalInput")
    d_grev = nc.dram_tensor("grev", [C, C], F32R, kind="ExternalInput")
    d_negall = nc.dram_tensor("negall", [C, C], F32R, kind="ExternalInput")
    d_negcol = nc.dram_tensor("negcol", [C, 4], F32R, kind="ExternalInput")
    d_onesrb = nc.dram_tensor("onesrb", [1, C], BF16, kind="ExternalInput")
    d_maskJ = nc.dram_tensor("maskj", [C, C], BF16, kind="ExternalInput")
    d_identb = nc.dram_tensor("identb", [128, 128], BF16, kind="ExternalInput")
    d_identf = nc.dram_tensor("identf", [128, 128], F32, kind="ExternalInput")

    d_out = nc.dram_tensor("out", [T, HID], BF16, kind="ExternalOutput")

    with tile.TileContext(nc) as tc:
        with (
            tc.tile_pool(name="persist", bufs=1) as pp,
            tc.tile_pool(name="wstage", bufs=2) as wsp,
            tc.tile_pool(name="xpad", bufs=2) as xpp,
            tc.tile_pool(name="scrA", bufs=2) as scA,
            tc.tile_pool(name="scrB", bufs=4) as scB,
            tc.tile_pool(name="scrC", bufs=4) as scC,
            tc.tile_pool(name="scrD", bufs=3) as scD,
            tc.tile_pool(name="states", bufs=8) as stp,
            tc.tile_pool(name="tiny", bufs=3) as tnp,
            tc.tile_pool(name="ps_big", bufs=2, space=MS.PSUM) as ps_big,
            tc.tile_pool(name="ps_cum", bufs=2, space=MS.PSUM) as ps_cum,
            tc.tile_pool(name="ps_sc", bufs=2, space=MS.PSUM) as ps_sc,
            tc.tile_pool(name="ps_tr", bufs=2, space=MS.PSUM) as ps_tr,
        ):
            # ---------- constants (scalar engine DMA queue) ----------
            def load_const(dram, shape, dtype=F32):
                t = pp.tile(shape, dtype, tag=dram.name + "_sb")
                nc.scalar.dma_start(t[:], dram[:])
                return t

            c_trineg = load_const(d_trineg, [C, C], F32R)
            c_gcpa = load_const(d_gcpa, [C, C], F32R)
            c_gcp63 = load_const(d_gcp63, [C, C], F32R)
            c_grev = load_const(d_grev, [C, C], F32R)
            c_negall = load_const(d_negall, [C, C], F32R)
            c_negcol = load_const(d_negcol, [C, 4], F32R)
            c_onesrb = load_const(d_onesrb, [1, C], BF16)
            c_maskJ = load_const(d_maskJ, [C, C], BF16)
            c_identb = load_const(d_identb, [128, 128], BF16)
            c_identf = load_const(d_identf, [128, 128], F32)
            c_wf2 = load_const(d_wf2, [DV, HL * M], BF16)
            c_wg2 = load_const(d_wg2, [DV, HL * DV], BF16)
            c_bg2 = load_const(d_bg2, [1, HL * DV], BF16)
            c_cdiag = load_const(d_cdiag, [128, 32, 128], BF16)
            c_eps6 = pp.tile([C, 1], F32, tag="c_eps6")
            nc.vector.memset(c_eps6[:], 1e-6)

            # ---------- X^T (interleaved with first weights) ----------
            xt_sb = pp.tile([128, NKT, T], BF16, tag="xt_sb")
            xtr = d_xt.rearrange("(k p) t -> k p t", p=128)
            w_ww = wsp.tile([128, NKT, HL * 128], BF16, tag="w_load")
            wwr = d_ww.rearrange("(k p) c -> k p c", p=128)
            for kt in range(NKT):
                nc.gpsimd.dma_start(w_ww[:, kt, :], wwr[kt])
                nc.sync.dma_start(xt_sb[:, kt, :], xtr[kt])

            # ---------- P1: projections + conv(diag-matmul) + silu ----------
            qT = pp.tile([128, HL, T], BF16, tag="qT")
            kT = pp.tile([128, HL, T], BF16, tag="kT")
            vTc = pp.tile([128, HL, T], BF16, tag="vTc")
            wTc = pp.tile([128, HL, T], BF16, tag="wTc")

            def project_conv(d_w, pi, out_tile, w_pre=None):
                if w_pre is not None:
                    w_sb = w_pre
                else:
                    w_sb = wsp.tile([128, NKT, HL * 128], BF16, tag="w_load")
                    wr = d_w.rearrange("(k p) c -> k p c", p=128)
                    for kt in range(NKT):
                        nc.gpsimd.dma_start(w_sb[:, kt, :], wr[kt])
                for ct in range(HL):
                    xpad = xpp.tile([128, T + KW - 1], BF16, tag="xpad")
                    nc.vector.memset(xpad[:, 0:KW - 1], 0.0)
                    for tt in range(2):
                        ps = ps_big.tile([128, 512], F32, tag="pp")
                        for kt in range(NKT):
                            nc.tensor.matmul(
                                ps[:],
                                w_sb[:, kt, ct * 128:(ct + 1) * 128],
                                xt_sb[:, kt, tt * 512:(tt + 1) * 512],
                                start=(kt == 0), stop=(kt == NKT - 1),
                            )
                        dst = xpad[:, KW - 1 + tt * 512: KW - 1 + (tt + 1) * 512]
                        if tt == 0:
                            nc.scalar.copy(dst, ps[:])
                        else:
                            nc.vector.tensor_copy(dst, ps[:])
                    for tt in range(2):
                        cps = ps_cum.tile([128, 512], F32, tag="cum")
                        for i in range(KW):
                            nc.tensor.matmul(
                                cps[:],
                                c_cdiag[:, (pi * HL + ct) * KW + i, :],
                                xpad[:, tt * 512 + i: tt * 512 + i + 512],
                                start=(i == 0), stop=(i == KW - 1))
                        nc.scalar.activation(
                            out_tile[:, ct, tt * 512:(tt + 1) * 512],
                            cps[:], AF.Silu)

            def proj128T(d_w, tag):
                out = pp.tile([128, T], BF16, tag=tag)
                w_sb = wsp.tile([128, NKT, 128], BF16, tag="w_load")
                wr = d_w.rearrange("(k p) c -> k p c", p=128)
                for kt in range(NKT):
                    nc.gpsimd.dma_start(w_sb[:, kt, :], wr[kt])
                for tt in range(2):
                    ps = ps_big.tile([128, 512], F32, tag="pp")
                    for kt in range(NKT):
                        nc.tensor.matmul(
                            ps[:], w_sb[:, kt, :],
                            xt_sb[:, kt, tt * 512:(tt + 1) * 512],
                            start=(kt == 0), stop=(kt == NKT - 1))
                    if tt == 0:
                        nc.scalar.copy(out[:, tt * 512:(tt + 1) * 512], ps[:])
                    else:
                        nc.vector.tensor_copy(out[:, tt * 512:(tt + 1) * 512],
                                              ps[:])
                return out

            project_conv(d_ww, 3, wTc, w_pre=w_ww)
            f1T = proj128T(d_wf1, "f1T")

            # ---------- beta ----------
            betaT = pp.tile([HL, T], F32, tag="betaT")
            wb_sb = wsp.tile([128, NKT, HL], BF16, tag="wb_load")
            wbr = d_wb.rearrange("(k p) c -> k p c", p=128)
            for kt in range(NKT):
                nc.gpsimd.dma_start(wb_sb[:, kt, :], wbr[kt])
            beta_ps = []
            for tt in range(2):
                ps = ps_big.tile([128, 512], F32, tag="pp")
                for kt in range(NKT):
                    nc.tensor.matmul(
                        ps[0:HL, :], wb_sb[:, kt, :],
                        xt_sb[:, kt, tt * 512:(tt + 1) * 512],
                        start=(kt == 0), stop=(kt == NKT - 1))
                beta_ps.append(ps)
            for tt in range(2):
                nc.scalar.activation(betaT[:, tt * 512:(tt + 1) * 512],
                                     beta_ps[tt][0:HL, :], AF.Sigmoid)
            beta_t = pp.tile([128, NCH, HL], F32, tag="beta_t")
            for n in range(NCH):
                ps = ps_big.tile([128, HL], F32, tag="pp")
                nc.tensor.matmul(ps[:], betaT[:, n * C:(n + 1) * C],
                                 c_identf[0:HL, 0:HL], is_transpose=True)
                nc.scalar.copy(beta_t[:, n, :], ps[:])

            # ---------- gpos = ln(1 + exp(-s)) ----------
            gpos_sb = pp.tile([128, NCH, HL * M], F32R, tag="gpos_sb")
            gpe_sb = pp.tile([128, NCH, HL * M], F32, tag="gpe_sb")
            for n in range(NCH):
                ps = ps_cum.tile([128, HL * M], F32, tag="cum")
                nc.tensor.matmul(ps[:], f1T[:, n * C:(n + 1) * C], c_wf2[:],
                                 start=True, stop=True)
                nc.scalar.activation(gpe_sb[:, n, :], ps[:], AF.Exp,
                                     scale=-1.0)
            nc.scalar.activation(gpos_sb[:], gpe_sb[:], AF.Ln, bias=1.0)

            # ---------- W transpose + l2norm + beta -> bw ----------
            bw2 = pp.tile([128, NCH, HL * M], BF16, tag="bw2")
            ssq_all = pp.tile([128, NCH, HL], F32, tag="ssq_all")
            lnr_all = pp.tile([128, NCH, HL], F32, tag="lnr_all")
            rs_all = pp.tile([128, NCH, HL], F32, tag="rs_all")
            rsb_all = pp.tile([128, NCH, HL], F32, tag="rsb_all")
            Wc_all = pp.tile([128, NCH, HL * M], BF16, tag="Wc_all")
            for n in range(NCH):
                for h in range(HL):
                    psw = ps_tr.tile([128, 512], BF16, tag="trb")
                    nc.tensor.transpose(psw[:, 0:128],
                                        wTc[:, h, n * C:(n + 1) * C],
                                        c_identb[:])
                    wc = Wc_all[:, n, h * M:(h + 1) * M]
                    nc.vector.tensor_copy(wc, psw[:, 0:128])
                    junk = scC.tile([128, 128], BF16, tag="junkw")
                    nc.vector.scalar_tensor_tensor(
                        junk[:], wc, 1.0, wc,
                        op0=ALU.mult, op1=ALU.mult,
                        accum_out=ssq_all[:, n, h:h + 1])
            nc.scalar.activation(lnr_all[:], ssq_all[:], AF.Ln, bias=c_eps6[:])
            nc.scalar.activation(rs_all[:], lnr_all[:], AF.Exp, scale=-0.5)
            nc.vector.tensor_mul(rsb_all[:], rs_all[:], beta_t[:])
            for n in range(NCH):
                for h in range(HL):
                    nc.vector.tensor_scalar_mul(
                        bw2[:, n, h * M:(h + 1) * M],
                        Wc_all[:, n, h * M:(h + 1) * M],
                        rsb_all[:, n, h:h + 1])

            # ---------- remaining projections ----------
            project_conv(d_wk, 1, kT)
            project_conv(d_wq, 0, qT)
            project_conv(d_wv, 2, vTc)

            # ---------- persisted phase-B products + states ----------
            den_sb = pp.tile([128, NCH, HL], F32, tag="den_sb")
            o_sb = pp.tile([128, NCH, HL, DV], BF16, tag="o_sb")
            oT_sb = pp.tile([128, HL, T], BF16, tag="oT_sb")
            Lam_all = pp.tile([128, NCH, HL * M], BF16, tag="Lam_all")
            EposA_all = pp.tile([128, NCH, HL * M], BF16, tag="EposA_all")
            Epos63_all = pp.tile([128, NCH, HL * M], BF16, tag="Epos63_all")
            s2_all = pp.tile([128, NCH, HL, 128], F32, tag="s2_all")
            EnegAT_all = pp.tile([128, NCH, HL, 128], BF16, tag="EnegAT_all")
            En63Tu_all = pp.tile([128, NCH, HL, 64], BF16, tag="En63Tu_all")
            Vc_all = pp.tile([128, NCH, HL, 128], BF16, tag="Vc_all")
            Skb_n = [[None, None] for _ in range(NCH)]
            Svb_n = [[None, None] for _ in range(NCH)]

            class Packer:
                def __init__(self, pool, tag, dtype, width, slots):
                    self.pool, self.tag, self.dtype = pool, tag, dtype
                    self.width, self.slots = width, slots
                    self.i = 0
                    self.cur = None

                def next(self, w=None):
                    w = w or self.width
                    if self.i % self.slots == 0:
                        self.cur = self.pool.tile(
                            [128, self.width * self.slots], self.dtype,
                            tag=self.tag)
                    off = (self.i % self.slots) * self.width
                    self.i += 1
                    return self.cur[:, off:off + w]

            trp = Packer(ps_tr, "trb", BF16, 128, 4)
            scp = Packer(ps_sc, "sc", F32, 128, 3)

            # ---------- phase B: state-independent chunk work + state chain ----------
            for n in range(NCH):
                t0 = n * C
                tsl = slice(t0, t0 + C)
                gsl = gpos_sb[:, n, :]
                gc_ps = ps_cum.tile([128, HL * M], F32, tag="cum")
                nc.tensor.matmul(gc_ps[:], c_trineg[:], gsl,
                                 start=True, stop=True)
                ga_ps = ps_cum.tile([128, HL * M], F32, tag="cum")
                nc.tensor.matmul(ga_ps[:], c_gcpa[:], gsl,
                                 start=True, stop=True)
                g6_ps = ps_cum.tile([128, HL * M], F32, tag="cum")
                nc.tensor.matmul(g6_ps[:], c_gcp63[:], gsl,
                                 start=True, stop=True)
                gr_ps = ps_cum.tile([128, HL * M], F32, tag="cum")
                nc.tensor.matmul(gr_ps[:], c_grev[:], gsl,
                                 start=True, stop=True)
                gl_ps = ps_cum.tile([128, HL * M], F32, tag="cum")
                nc.tensor.matmul(gl_ps[:], c_negall[:], gsl,
                                 start=True, stop=True)
                lcc = [None, None]
                if n < NCH - 1:
                    for h in range(HL):
                        hs = slice(h * M, (h + 1) * M)
                        lcc_ps = ps_big.tile([128, 4], F32, tag="pp")
                        nc.tensor.matmul(lcc_ps[0:M, :], gsl[:, hs],
                                         c_negcol[:], start=True, stop=True)
                        lcv = tnp.tile([M, 1], F32, tag="lcc_sb")
                        nc.scalar.activation(lcv[:], lcc_ps[0:M, 0:1], AF.Exp)
                        lcc[h] = lcv
                Lam = Lam_all[:, n, :]
                nc.scalar.activation(Lam, gc_ps[:], AF.Exp)
                EposA = EposA_all[:, n, :]
                nc.scalar.activation(EposA, ga_ps[:], AF.Exp)
                EnegAe = scB.tile([128, HL * M], BF16, tag="EnegAe")
                nc.scalar.activation(EnegAe[:], ga_ps[:], AF.Exp, scale=-1.0)
                Epos63 = Epos63_all[:, n, :]
                nc.scalar.activation(Epos63, g6_ps[:], AF.Exp)
                Eneg63e = scB.tile([128, HL * M], BF16, tag="Eneg63e")
                nc.scalar.activation(Eneg63e[:], g6_ps[:], AF.Exp, scale=-1.0)
                Ereve = scB.tile([128, HL * M], BF16, tag="Ereve")
                nc.scalar.activation(Ereve[:], gr_ps[:], AF.Exp)
                LamCb = scA.tile([128, HL * M], F32, tag="LamCb")
                nc.scalar.activation(LamCb[:], gl_ps[:], AF.Exp)
                EnegA = scB.tile([128, HL * M], BF16, tag="EnegA")
                nc.gpsimd.tensor_mul(EnegA[:], EnegAe[:], bw2[:, n, :])
                Eneg63 = scB.tile([128, HL * M], BF16, tag="Eneg63")
                nc.gpsimd.tensor_mul(Eneg63[:], Eneg63e[:], bw2[:, n, :])
                Kdec = scB.tile([128, HL * M], BF16, tag="Kdec")
                nc.gpsimd.tensor_mul(Kdec[:], Ereve[:], bw2[:, n, :])

                for h in range(HL):
                    hs = slice(h * M, (h + 1) * M)
                    psk = trp.next()
                    nc.tensor.transpose(psk, kT[:, h, tsl], c_identb[:])
                    Kc = scC.tile([128, 128], BF16, tag="Kc")
                    nc.scalar.copy(Kc[:], psk)
                    psv = trp.next()
                    nc.tensor.transpose(psv, vTc[:, h, tsl], c_identb[:])
                    nc.vector.tensor_copy(Vc_all[:, n, h, :], psv)
                    pse = trp.next()
                    nc.tensor.transpose(pse, EnegA[:, hs], c_identb[:])
                    nc.scalar.copy(EnegAT_all[:, n, h, :], pse)
                    ps6 = trp.next(64)
                    nc.tensor.transpose(ps6, Eneg63[0:64, hs],
                                        c_identb[0:64, 0:64])
                    nc.vector.tensor_copy(En63Tu_all[:, n, h, :], ps6)
                    pt_ps = scp.next()
                    nc.tensor.matmul(pt_ps, kT[:, h, tsl], qT[:, h, tsl],
                                     start=True, stop=True)
                    Ptm = scC.tile([128, 128], BF16, tag="Ptm")
                    nc.vector.scalar_tensor_tensor(
                        Ptm[:], pt_ps, SCALE, c_maskJ[:],
                        op0=ALU.mult, op1=ALU.mult)
                    intra_ps = scp.next()
                    nc.tensor.matmul(intra_ps, Ptm[:], Eneg63[:, hs],
                                     start=True, stop=True)
                    nc.vector.tensor_mul(s2_all[:, n, h, :], intra_ps,
                                         Epos63[:, hs])
                    if n < NCH - 1:
                        skk_ps = ps_big.tile([128, 128], F32, tag="pp")
                        nc.tensor.matmul(skk_ps[:], Kc[:], Kdec[:, hs],
                                         start=True, stop=True)
                        svk_ps = ps_big.tile([128, 128], F32, tag="pp")
                        nc.tensor.matmul(svk_ps[:], Kdec[:, hs],
                                         Vc_all[:, n, h, :],
                                         start=True, stop=True)
                        Skb_new = stp.tile([128, 128], BF16, tag=f"Skb{h}")
                        Svb_new = stp.tile([128, 128], BF16, tag=f"Svb{h}")
                        if n == 0:
                            nc.vector.tensor_copy(Skb_new[:], skk_ps[:])
                            nc.vector.tensor_copy(Svb_new[:], svk_ps[:])
                        else:
                            skt = scD.tile([128, 128], BF16, tag="skt")
                            nc.vector.tensor_mul(skt[:], Skb_n[n - 1][h][:],
                                                 LamCb[:, hs])
                            nc.vector.tensor_add(Skb_new[:], skt[:],
                                                 skk_ps[:])
                            nc.vector.scalar_tensor_tensor(
                                Svb_new[:], Svb_n[n - 1][h][:], lcc[h][:],
                                svk_ps[:], op0=ALU.mult, op1=ALU.add)
                        Skb_n[n][h] = Skb_new
                        Svb_n[n][h] = Svb_new

            # ---------- g1 / gate (needed only by the epilogue) ----------
            g1T = proj128T(d_wg1, "g1T")
            gate_sb = pp.tile([128, NCH, HL * DV], BF16, tag="gate_sb")
            gate_ps = []
            for n in range(NCH):
                ps = ps_cum.tile([128, HL * DV], F32, tag="cum")
                nc.tensor.matmul(ps[:], g1T[:, n * C:(n + 1) * C], c_wg2[:],
                                 start=True, stop=False)
                nc.tensor.matmul(ps[:], c_onesrb[:], c_bg2[:],
                                 start=False, stop=True)
                gate_ps.append(ps)
            for n in range(NCH):
                nc.scalar.activation(gate_sb[:, n, :], gate_ps[n][:],
                                     AF.Sigmoid)

            # ---------- phase C: softmax waves ----------
            for n in range(NCH):
                t0 = n * C
                tsl = slice(t0, t0 + C)
                for h in range(HL):
                    hs = slice(h * M, (h + 1) * M)
                    if n == 0:
                        sS = s2_all[:, 0, h, :]
                    else:
                        qs_ps = ps_cum.tile([128, 128], F32, tag="cum")
                        nc.tensor.matmul(qs_ps[:], qT[:, h, tsl],
                                         Skb_n[n - 1][h][:],
                                         start=True, stop=True)
                        s1 = scD.tile([128, 128], F32, tag="s1")
                        nc.vector.scalar_tensor_tensor(
                            s1[:], qs_ps[:], SCALE, Lam_all[:, n, hs],
                            op0=ALU.mult, op1=ALU.mult)
                        sS = scD.tile([128, 128], F32, tag="sS")
                        nc.vector.tensor_add(sS[:], s1[:], s2_all[:, n, h, :])
                    nmx = tnp.tile([128, 1], F32, tag="nmx")
                    nc.vector.tensor_reduce(nmx[:], sS[:], AX.X, ALU.max,
                                            negate=True)
                    pexp = scC.tile([128, 128], BF16, tag="pexp")
                    nc.scalar.activation(pexp[:], sS[:], AF.Exp, bias=nmx[:],
                                         accum_out=den_sb[:, n, h:h + 1])
                    aL = scC.tile([128, 128], BF16, tag="aL")
                    nc.vector.tensor_mul(aL[:], pexp[:], Lam_all[:, n, hs])
                    aEA = scC.tile([128, 128], BF16, tag="aEA")
                    nc.vector.tensor_mul(aEA[:], pexp[:], EposA_all[:, n, hs])
                    aE63u = scC.tile([64, 128], BF16, tag="aE63u")
                    nc.gpsimd.tensor_mul(aE63u[:], pexp[64:128, :],
                                         Epos63_all[64:128, n, hs])
                    psl = trp.next()
                    nc.tensor.transpose(psl, aL[:], c_identb[:])
                    aLT = scC.tile([128, 128], BF16, tag="aLT")
                    nc.scalar.copy(aLT[:], psl)
                    psa = trp.next()
                    nc.tensor.transpose(psa, aEA[:], c_identb[:])
                    aEAT = scC.tile([128, 128], BF16, tag="aEAT")
                    nc.vector.tensor_copy(aEAT[:], psa)
                    ps63 = trp.next(64)
                    nc.tensor.transpose(ps63, aE63u[:],
                                        c_identb[0:64, 0:64])
                    aE63uT = scC.tile([128, 64], BF16, tag="aE63uT")
                    nc.scalar.copy(aE63uT[:], ps63)
                    rt_ps = scp.next()
                    nc.tensor.matmul(rt_ps[0:64, 0:64],
                                     EnegAT_all[:, n, h, 0:64],
                                     aEAT[:, 0:64], start=True, stop=True)
                    nc.tensor.matmul(rt_ps[64:128, 64:128],
                                     EnegAT_all[:, n, h, 64:128],
                                     aEAT[:, 64:128], start=True, stop=True)
                    nc.tensor.matmul(rt_ps[0:64, 64:128],
                                     En63Tu_all[:, n, h, :],
                                     aE63uT[:], start=True, stop=True)
                    Rmt = scC.tile([128, 128], BF16, tag="Rmt")
                    nc.vector.memset(Rmt[64:128, 0:64], 0.0)
                    nc.vector.tensor_mul(Rmt[0:64, :], rt_ps[0:64, :],
                                         c_maskJ[0:64, :])
                    nc.vector.tensor_mul(Rmt[64:128, 64:128],
                                         rt_ps[64:128, 64:128],
                                         c_maskJ[64:128, 64:128])
                    o_ps = scp.next()
                    if n == 0:
                        nc.tensor.matmul(o_ps, Rmt[:], Vc_all[:, n, h, :],
                                         start=True, stop=True)
                    else:
                        nc.tensor.matmul(o_ps, aLT[:], Svb_n[n - 1][h][:],
                                         start=True, stop=False)
                        nc.tensor.matmul(o_ps, Rmt[:], Vc_all[:, n, h, :],
                                         start=False, stop=True)
                    nc.scalar.copy(o_sb[:, n, h, :], o_ps)

            # ---------- epilogue in two halves + Wo (tail overlap) ----------
            osum_all = pp.tile([128, NCH, HL], F32, tag="osum_all")
            d2_all = pp.tile([128, NCH, HL], F32, tag="d2_all")
            ox_all = pp.tile([128, NCH, HL], F32, tag="ox_all")
            orm_all = pp.tile([128, NCH, HL], F32, tag="orm_all")
            orr_all = pp.tile([128, NCH, HL], F32, tag="orr_all")

            wo_sb = pp.tile([128, HL, HID], BF16, tag="wo_sb")
            wor = d_wo.rearrange("(h p) o -> h p o", p=128)
            for h in range(HL):
                nc.scalar.dma_start(wo_sb[:, h, :], wor[h])

            HN = NCH // 2
            for half in range(2):
                n0, n1 = half * HN, (half + 1) * HN
                for n in range(n0, n1):
                    for h in range(HL):
                        junk2 = scC.tile([128, 128], BF16, tag="junko")
                        nc.vector.scalar_tensor_tensor(
                            junk2[:], o_sb[:, n, h, :], 1.0, o_sb[:, n, h, :],
                            op0=ALU.mult, op1=ALU.mult,
                            accum_out=osum_all[:, n, h:h + 1])
                sl = slice(n0, n1)
                nc.vector.scalar_tensor_tensor(
                    d2_all[:, sl, :], den_sb[:, sl, :], EPS, den_sb[:, sl, :],
                    op0=ALU.mult, op1=ALU.mult)
                nc.vector.scalar_tensor_tensor(
                    ox_all[:, sl, :], osum_all[:, sl, :], 1.0 / DV,
                    d2_all[:, sl, :], op0=ALU.mult, op1=ALU.add)
                nc.scalar.activation(orm_all[:, sl, :], ox_all[:, sl, :],
                                     AF.Ln)
                nc.scalar.activation(orr_all[:, sl, :], orm_all[:, sl, :],
                                     AF.Exp, scale=-0.5)
                for n in range(n0, n1):
                    for h in range(HL):
                        of = scC.tile([128, 128], BF16, tag="of")
                        nc.vector.scalar_tensor_tensor(
                            of[:], o_sb[:, n, h, :], orr_all[:, n, h:h + 1],
                            gate_sb[:, n, h * DV:(h + 1) * DV],
                            op0=ALU.mult, op1=ALU.mult)
                        pso = trp.next()
                        nc.tensor.transpose(pso, of[:], c_identb[:])
                        nc.scalar.copy(oT_sb[:, h, n * C:(n + 1) * C], pso)
                for tt in range(n0, n1):
                    for cl in range(4):
                        ps = ps_big.tile([128, 512], F32, tag="pp")
                        for h in range(HL):
                            nc.tensor.matmul(
                                ps[:], oT_sb[:, h, tt * C:(tt + 1) * C],
                                wo_sb[:, h, cl * 512:(cl + 1) * 512],
                                start=(h == 0), stop=(h == HL - 1))
                        osb = scC.tile([128, 512], BF16, tag="outsb")
                        if cl % 2 == 0:
                            nc.scalar.copy(osb[:], ps[:])
                        else:
                            nc.vector.tensor_copy(osb[:], ps[:])
                        nc.sync.dma_start(
                            d_out[tt * 128:(tt + 1) * 128,
                                  cl * 512:(cl + 1) * 512],
                            osb[:])
    nc.compile()
    return nc


def _host_inputs(inputs):
    f32 = np.float32
    bf16 = ml_dtypes.bfloat16
    X = np.ascontiguousarray(np.asarray(inputs["hidden_states"], f32)[0])
    XT = np.ascontiguousarray(X.T).astype(bf16)

    trineg = np.triu(np.full((C, C), -1.0, f32))
    nb1a = np.zeros((C, C), f32)
    nb1a[0:32, 0:64] = -1.0
    nb1a[0:96, 64:128] = -1.0
    neg63 = np.zeros((C, C), f32)
    neg63[0:64, :] = -1.0
    negall = np.full((C, C), -1.0, f32)
    negcol = np.full((C, 4), -1.0, f32)
    onesr = np.ones((1, C), f32)
    maskJ = np.triu(np.ones((C, C), f32))
    ident = np.eye(128, dtype=f32)
    gcpa = trineg - nb1a
    gcp63 = trineg - neg63
    grev_m = negall - trineg

    Wo_full = np.asarray(inputs["Wo"], f32) * np.tile(
        np.asarray(inputs["norm_w"], f32), H)[:, None]

    def cdiag_for(c):
        out = np.zeros((128, 32, 128), f32)
        idx = np.arange(128)
        for pi, key in enumerate(["cq", "ck", "cv", "cv"]):
            cw = np.asarray(inputs[key], f32)
            for ct in range(HL):
                ch = (c * HL + ct) * 128
                for i in range(KW):
                    out[idx, (pi * HL + ct) * KW + i, idx] = cw[ch + idx, i]
        return out.astype(bf16)

    in_maps = []
    for c in range(8):
        hsl = slice(c * HL * 128, (c + 1) * HL * 128)
        bsl = slice(c * HL, (c + 1) * HL)
        m = {
            "xt": XT,
            "wq": np.asarray(inputs["Wq"], f32)[:, hsl].astype(bf16),
            "wk": np.asarray(inputs["Wk"], f32)[:, hsl].astype(bf16),
            "wv": np.asarray(inputs["Wv"], f32)[:, hsl].astype(bf16),
            "ww": np.asarray(inputs["Ww"], f32)[:, hsl].astype(bf16),
            "wf1": np.asarray(inputs["Wf1"], f32).astype(bf16),
            "wg1": np.asarray(inputs["Wg1"], f32).astype(bf16),
            "wb": np.asarray(inputs["Wb"], f32)[:, bsl].astype(bf16),
            "wf2": np.ascontiguousarray(
                np.asarray(inputs["Wf2"], f32)[:, hsl]).astype(bf16),
            "wg2": np.ascontiguousarray(
                np.asarray(inputs["Wg2"], f32)[:, hsl]).astype(bf16),
            "bg2": np.ascontiguousarray(
                np.asarray(inputs["bg2"], f32)[None, hsl]).astype(bf16),
            "wo": np.ascontiguousarray(Wo_full[hsl]).astype(bf16),
            "cdiag": cdiag_for(c),
            "trineg": trineg, "gcpa": gcpa, "gcp63": gcp63, "grev": grev_m,
            "negall": negall, "negcol": negcol,
            "onesrb": onesr.astype(bf16), "maskj": maskJ.astype(bf16),
            "identb": ident.astype(bf16), "identf": ident,
        }
        in_maps.append(m)
    return in_maps


def kernel(_trace=False, **inputs):
    if "nc" not in _CACHE:
        _CACHE["nc"] = _build_nc()
    nc = _CACHE["nc"]
    in_maps = _host_inputs(inputs)
    res = run_bass_kernel_spmd(nc, in_maps, core_ids=list(range(8)),
                               trace=_trace)
    _CACHE["last_result"] = res
    out = np.zeros((T, HID), np.float32)
    for r in res.results:
        out += np.asarray(r["out"], np.float32)
    return out.reshape(B, T, HID)
